# revision 28
# baseline (speedup 1.0000x reference)
"""MultiHeadAttention (qk-LayerNorm + RoPE) Trainium2 kernel, 8 NeuronCores.

Sharding: batch (4) x head-group (2x8 heads). Core c handles batch c//2,
heads 8*(c%2) .. 8*(c%2)+7. Each core computes QKV projections for its
batch restricted to its head group, per-head LayerNorm + rotary embedding,
attention, and a partial output projection over its 512 context channels.
The two partial o_proj results per batch are summed on the host (the
"unshard" step), which keeps the device program collective-free: no NEFF
entry barrier, no ReduceScatter tail.

Dataflow per core (all matmul operands bf16, PSUM accumulation fp32):
  Phase 1: per 128-token tile: QKV projections (bf16, x and per-ck weight
    tiles DMA'd in interleaved order so matmuls start early), LayerNorm
    stats via per-head reductions (merged across two token tiles),
    LN+rope applied in bf16, q/k transposed to [d, t] layout via PE
    transposes (two heads per 128x128 transpose).
  Phase 2: per head pair (row groups 0:64 / 64:128 of the packed q/k
    tiles): scores for both heads concurrently (distinct PE row groups),
    one exp ACT op per j covering both heads [128, 2048], ctx accumulated
    per 512-token half into 1-bank PSUM tiles (m=0 interleaved into the
    j-loop, m=1 as a dense burst over the retained exp tiles) with a ones
    column appended to v so the softmax denominator falls out of the same
    matmul. Normalization per half: DVE copy of the denominator row to
    partition 0, gpsimd partition_broadcast, reciprocal_approx_fast, one
    multiply per head; the odd head is moved to partitions 64:127 with a
    cross-quadrant stream_shuffle so o_proj runs K=128 per head pair.
  Phase 3: o_proj per token tile (4 accumulating K=128 matmuls, weights
    reused across the two 512-column halves), fp32 partial DMA'd to DRAM.
"""
import sys

for _p in ("/opt/trn_rl_repo", "/root/.axon_site", "/root/.axon_site/_ro/trn_rl_repo",
           "/root/.axon_site/_ro/pypackages"):
    if _p not in sys.path:
        sys.path.append(_p)

import numpy as np

import concourse.bass as bass
import concourse.tile as tile
from concourse import bacc, mybir
from concourse.bass_utils import run_bass_kernel_spmd
from concourse.masks import make_identity

F32 = mybir.dt.float32
F32R = mybir.dt.float32r
BF16 = mybir.dt.bfloat16
P = 128
B, L, C, H, D = 4, 1024, 1024, 16, 64
HC = 8          # heads per core
NPR = HC // 2   # head pairs per core
CG = HC * D     # 512 context channels per core
NT = L // P     # 8 token tiles
NCK = C // P    # 8 contraction tiles
THETA = 50000.0
EPS = 1e-5

_NC_CACHE = {}
# dummy keep-warm matmul counts (fill PE idle so the HAM clock gate stays
# at K=8/8; targets are PSUM slivers cleared by the next start=True group)
WARM1, WARM2, WARM2E, WARM3, WARM3PRE = 0, 0, 0, 0, 0


def _build_nc():
    nc = bacc.Bacc("TRN2", target_bir_lowering=False, debug=False, num_devices=8)

    xT_d = nc.dram_tensor("xT", [P, NT, NCK, P], BF16, kind="ExternalInput")
    wqT_d = nc.dram_tensor("wqT", [C, CG], BF16, kind="ExternalInput")
    wkT_d = nc.dram_tensor("wkT", [C, CG], BF16, kind="ExternalInput")
    wvT_d = nc.dram_tensor("wvT", [C, CG], BF16, kind="ExternalInput")
    woT_d = nc.dram_tensor("woT", [NPR, P, C], BF16, kind="ExternalInput")
    aq_d = nc.dram_tensor("aq", [P, NT, D], BF16, kind="ExternalInput")
    bq_d = nc.dram_tensor("bq", [P, NT, D], BF16, kind="ExternalInput")
    ak_d = nc.dram_tensor("ak", [P, NT, D], BF16, kind="ExternalInput")
    bk_d = nc.dram_tensor("bk", [P, NT, D], BF16, kind="ExternalInput")
    out_d = nc.dram_tensor("out", [L, C], F32, kind="ExternalOutput")

    with tile.TileContext(nc) as tc:
        with (
            tc.tile_pool(name="const", bufs=1) as constp,
            tc.tile_pool(name="w", bufs=1) as wpool,
            tc.tile_pool(name="big", bufs=1) as bigp,
            tc.tile_pool(name="scr", bufs=2) as scrp,
            tc.tile_pool(name="rope", bufs=2) as ropep,
            tc.tile_pool(name="stat", bufs=2) as statp,
            tc.tile_pool(name="exp", bufs=1) as expp,
            tc.tile_pool(name="den", bufs=2) as denp,
            tc.tile_pool(name="fin", bufs=2) as finp,
        ):
            ident = constp.tile([P, P], BF16)
            make_identity(nc, ident)
            eps_t = constp.tile([P, 1], F32)
            nc.vector.memset(eps_t[:], EPS)

            a2_t = constp.tile([P, 2, NT, D], BF16)
            b2_t = constp.tile([P, 2, NT, D], BF16)

            # x resident in SBUF, tile-major. DMA order: x tile 0, all wq,
            # all wk, x tile 1, all wv, x tiles 2..7 — so tile 0's q stats
            # (the head of the DVE pipeline) are ready after ~1.3MB of
            # traffic instead of the full 5MB
            xt_all = bigp.tile([P, NT, NCK, P], BF16)
            wq_t, wk_t, wv_t = [], [], []

            def _w_dmas(lst, nm, d_):
                for ck in range(NCK):
                    t_ = wpool.tile([P, CG], BF16, tag=f"{nm}{ck}", name=f"{nm}{ck}")
                    nc.sync.dma_start(
                        t_[:],
                        d_.ap().rearrange("(k p) o -> p k o", p=P)[:, ck, :])
                    lst.append(t_)

            nc.sync.dma_start(xt_all[:, 0], xT_d.ap()[:, 0])
            _w_dmas(wq_t, "wq", wqT_d)
            _w_dmas(wk_t, "wk", wkT_d)
            nc.sync.dma_start(xt_all[:, 1], xT_d.ap()[:, 1])
            _w_dmas(wv_t, "wv", wvT_d)
            for ti in range(2, NT):
                nc.sync.dma_start(xt_all[:, ti], xT_d.ap()[:, ti])

            nc.sync.dma_start(a2_t[:, 0, :, :], aq_d.ap())
            nc.sync.dma_start(a2_t[:, 1, :, :], ak_d.ap())
            nc.sync.dma_start(b2_t[:, 0, :, :], bq_d.ap())
            nc.sync.dma_start(b2_t[:, 1, :, :], bk_d.ap())

            # v with a ones column appended per head: [s_tile, j, head, 65]
            v_sb = bigp.tile([P, NT, HC, D + 1], BF16)
            nc.vector.memset(
                v_sb[:, :, :, D:D + 1].rearrange("p t h o -> p (t h) o"), 1.0)
            def warm(n, target):
                for _ in range(n):
                    nc.tensor.matmul(target[0:16, 0:16], xt_all[:, 0, 0, 16:32],
                                     xt_all[:, 0, 0, 0:16], start=True, stop=True)


            qT_pack = bigp.tile([P, NPR, L], BF16)
            kT_pack = bigp.tile([P, NPR, L], BF16)
            # ctx packed two heads per 128 partitions: [128, pair, L]
            ctxT2 = bigp.tile([P, NPR, L], BF16)

            # ---------------- Phase 1: QKV + LN + RoPE + transpose ----------
            # processed two token tiles per group: the QKV matmuls and the
            # PSUM-reading ops (reduces, squares, t1) run per tile, the rest
            # of the LN/rope arithmetic runs as merged [P, 2, 2, HC, *] DVE
            # ops to amortize per-op overhead and pipeline drains
            with tc.tile_pool(name="ps1", bufs=2, space="PSUM") as ps1, \
                 tc.tile_pool(name="pst", bufs=2, space="PSUM") as pst:
                for g in range(NT // 2):
                    stats = statp.tile([P, 2, 4, HC], F32, tag="stats")
                    qk_sb = scrp.tile([P, 2, 2, HC, D], BF16, tag="qk_sb")
                    t1 = qk_sb  # LN-apply runs in place on the bf16 copy
                    psqks = []
                    for s in range(2):
                        ti = 2 * g + s
                        psq = ps1.tile([P, CG], F32, tag="psq", name="psq")
                        psk = ps1.tile([P, CG], F32, tag="psk", name="psk")
                        psv = ps1.tile([P, CG], F32, tag="psv", name="psv")
                        psqks.append((psq, psk))
                        for ps_, w_ in ((psq, wq_t), (psk, wk_t), (psv, wv_t)):
                            for ck in range(NCK):
                                nc.tensor.matmul(
                                    ps_[:], xt_all[:, ti, ck, :], w_[ck][:],
                                    start=(ck == 0), stop=(ck == NCK - 1))

                        # v straight to SBUF (bf16); ACT to keep DVE free
                        nc.scalar.copy(
                            v_sb[:, ti, :, 0:D],
                            psv[:].rearrange("p (h d) -> p h d", d=D))

                        # sums / sums of squares per (token, q/k, head);
                        # q/k also copied to bf16 SBUF (ACT) so the PSUM
                        # banks free early and the rope chain runs at the
                        # 2x bf16 DVE rate
                        for i, ps_ in enumerate((psq, psk)):
                            nc.vector.reduce_sum(
                                stats[:, s, 2 * i, :],
                                ps_[:].rearrange("p (h d) -> p h d", d=D),
                                axis=mybir.AxisListType.X)
                            nc.scalar.copy(qk_sb[:, s, i], ps_[:].rearrange(
                                "p (h d) -> p h d", d=D))
                            sq = scrp.tile([P, CG], F32, tag="sq")
                            nc.scalar.square(sq[:], ps_[:])
                            nc.vector.reduce_sum(
                                stats[:, s, 2 * i + 1, :],
                                sq[:].rearrange("p (h d) -> p h d", d=D),
                                axis=mybir.AxisListType.X)
                    mu2 = statp.tile([P, 2, 4, HC], F32, tag="mu2")
                    nc.vector.tensor_scalar_mul(mu2[:], stats[:], 1.0 / D)
                    var = statp.tile([P, 2, 2, HC], F32, tag="var")
                    nc.vector.tensor_mul(var[:], mu2[:, :, 0::2, :], mu2[:, :, 0::2, :])
                    nc.vector.tensor_sub(var[:], mu2[:, :, 1::2, :], var[:])
                    std = statp.tile([P, 2, 2, HC], F32, tag="std")
                    nc.scalar.activation(std[:], var[:],
                                         mybir.ActivationFunctionType.Sqrt,
                                         bias=eps_t[:])
                    inv = statp.tile([P, 2, 2, HC], F32, tag="inv")
                    nc.vector.reciprocal(inv[:], std[:])
                    invh = statp.tile([P, 2, 2, HC], BF16, tag="invh")
                    nc.vector.tensor_copy(invh[:], inv[:])
                    shifth = statp.tile([P, 2, 2, HC], BF16, tag="shifth")
                    nc.vector.tensor_mul(shifth[:], mu2[:, :, 0::2, :], inv[:])

                    h_ = D // 2
                    for s in range(2):
                        ti = 2 * g + s
                        inv_b = invh[:, s].rearrange("p i h -> p i h ()").to_broadcast(
                            (P, 2, HC, D))
                        sh_b = shifth[:, s].rearrange("p i h -> p i h ()").to_broadcast(
                            (P, 2, HC, D))
                        a_b = a2_t[:, :, ti, :].rearrange(
                            "p i d -> p i () d").to_broadcast((P, 2, HC, D))
                        nc.vector.tensor_mul(t1[:, s], t1[:, s], inv_b)
                        nc.vector.tensor_sub(t1[:, s], t1[:, s], sh_b)
                        rope = ropep.tile([P, 2, HC, D], BF16, tag=f"rope{s}")
                        nc.vector.tensor_mul(rope[:], t1[:, s], a_b)
                        r2 = scrp.tile([P, 2, HC, D], BF16, tag=f"r2{s}")
                        nc.vector.tensor_mul(
                            r2[:, :, :, 0:h_], t1[:, s, :, :, h_:D],
                            b2_t[:, :, ti, 0:h_].rearrange(
                                "p i d -> p i () d").to_broadcast((P, 2, HC, h_)))
                        nc.vector.tensor_mul(
                            r2[:, :, :, h_:D], t1[:, s, :, :, 0:h_],
                            b2_t[:, :, ti, h_:D].rearrange(
                                "p i d -> p i () d").to_broadcast((P, 2, HC, h_)))
                        nc.vector.tensor_add(rope[:], rope[:], r2[:])
                        for i, dstpack in ((0, qT_pack), (1, kT_pack)):
                            for pr in range(NPR):
                                ps_t = pst.tile([P, P], BF16)
                                nc.tensor.transpose(
                                    ps_t[:],
                                    rope[:, i, 2 * pr:2 * pr + 2, :].rearrange(
                                        "p h d -> p (h d)"),
                                    ident[:])
                                nc.scalar.copy(dstpack[:, pr, bass.ts(ti, P)], ps_t[:])
                    warm(WARM1, psqks[0][0])

            # o_proj weights early: reuses the per-ck wq slots (dead after
            # phase 1); packed per head pair [128, C] to match ctxT2
            wo_l = []
            for pr in range(NPR):
                wo_p = wpool.tile([P, C], BF16, tag=f"wq{pr}", name=f"wo{pr}")
                nc.sync.dma_start(wo_p[:], woT_d.ap()[pr, :, :])
                wo_l.append(wo_p)

            # ---------------- Phase 2: attention per head pair --------------
            # ctx accumulates per 512-token half into 1-bank PSUM tiles:
            # m=0 interleaved into the scores/exp j-loop, m=1 as a dense
            # matmul burst afterwards (all exp tiles are kept in SBUF).
            # Each half normalizes independently, so no pair-boundary
            # barrier on PSUM and the PE never idles long enough for the
            # HAM clock gate to re-throttle.
            with tc.tile_pool(name="pss", bufs=1, space="PSUM") as pssp, \
                 tc.tile_pool(name="psc", bufs=1, space="PSUM") as pscp:
                shuffle_ident = list(range(32))

                def normalize(pr, head, m, psc_):
                    den = denp.tile([1, 512], F32, tag=f"den{head}{m}")
                    nc.vector.tensor_copy(den[0:1, :], psc_[D:D + 1, :])
                    rbr = denp.tile([D, 512], F32, tag=f"rbr{head}{m}")
                    nc.gpsimd.partition_broadcast(rbr[:], den[0:1, :])
                    rb = denp.tile([D, 512], F32, tag=f"rb{head}{m}")
                    nc.vector.reciprocal_approx_fast(rb[:], rbr[:])
                    if head == 0:
                        nc.vector.tensor_mul(
                            ctxT2[0:D, pr, bass.ts(m, 512)], psc_[0:D, :], rb[:])
                    else:
                        tmpB = denp.tile([D, 512], BF16, tag=f"tmpB{m}")
                        nc.vector.tensor_mul(tmpB[:], psc_[0:D, :], rb[:])
                        nc.vector.stream_shuffle(
                            ctxT2[D:2 * D, pr, bass.ts(m, 512)], tmpB[:],
                            shuffle_ident)

                for pr in range(NPR):
                    hA, hB = 2 * pr, 2 * pr + 1
                    psc0 = [pscp.tile([D + 1, 512], F32, tag=f"pc{h}0",
                                      name=f"pc{h}0") for h in range(2)]
                    psc1 = [pscp.tile([D + 1, 512], F32, tag=f"pc{h}1",
                                      name=f"pc{h}1") for h in range(2)]
                    exps = []
                    for j in range(NT):
                        pss = pssp.tile([P, 2, 2, 512], F32, tag="pss")
                        for m in range(2):
                            for half in range(2):
                                nc.tensor.matmul(
                                    pss[:, half, m, :],
                                    kT_pack[half * D:(half + 1) * D, pr, bass.ts(j, P)],
                                    qT_pack[half * D:(half + 1) * D, pr, bass.ts(m, 512)],
                                    start=True, stop=True)
                        expAB = expp.tile([P, 2, 2, 512], BF16, tag=f"expAB{j}",
                                          name=f"expAB{j}")
                        nc.scalar.activation(expAB[:], pss[:],
                                             mybir.ActivationFunctionType.Exp,
                                             scale=float(D) ** -0.5)
                        exps.append(expAB)
                        # ctx for both halves of the previous j, interleaved
                        # with the scores/exp pipeline
                        if j >= 1:
                            for head, h in ((0, hA), (1, hB)):
                                for m, psc_ in ((0, psc0), (1, psc1)):
                                    nc.tensor.matmul(
                                        psc_[head][:], v_sb[:, j - 1, h, :],
                                        exps[j - 1][:, head, m, :],
                                        start=(j - 1 == 0), stop=(j - 1 == NT - 1))
                            warm(WARM2, psc1[0])
                    for head, h in ((0, hA), (1, hB)):
                        for m, psc_ in ((0, psc0), (1, psc1)):
                            nc.tensor.matmul(
                                psc_[head][:], v_sb[:, NT - 1, h, :],
                                exps[NT - 1][:, head, m, :],
                                start=False, stop=True)
                    for head in range(2):
                        normalize(pr, head, 0, psc0[head])
                    for head in range(2):
                        normalize(pr, head, 1, psc1[head])

            # ---------------- Phase 3: output projection --------------------
            with tc.tile_pool(name="pso", bufs=2, space="PSUM") as psop:
                for ti in range(NT):
                    pso = psop.tile([P, C], F32, name="pso")
                    for pr in range(NPR):
                        for m in range(2):
                            nc.tensor.matmul(
                                pso[:, bass.ts(m, 512)],
                                ctxT2[:, pr, bass.ts(ti, P)],
                                wo_l[pr][:, bass.ts(m, 512)],
                                start=(pr == 0), stop=(pr == NPR - 1))
                    out_sb = finp.tile([P, C], F32, tag="out", name="out_sb")
                    if ti % 2 == 0:
                        nc.scalar.copy(out_sb[:], pso[:])
                    else:
                        nc.vector.tensor_copy(out_sb[:], pso[:])
                    nc.sync.dma_start(out_d.ap()[bass.ts(ti, P), :], out_sb[:])

    nc.compile()
    return nc


def _rope_tables(w, b):
    """A[t,d], B[t,d] with the rotate-half sign folded into B."""
    inv_freq = 1.0 / THETA ** (np.arange(0, D, 2, dtype=np.float64) / D)
    freqs = np.arange(L, dtype=np.float64)[:, None] * inv_freq[None, :]
    freqs = np.concatenate([freqs, freqs], axis=1)           # [L, D]
    cos, sin = np.cos(freqs), np.sin(freqs)
    w = w.astype(np.float64)
    w_rot = np.concatenate([w[D // 2:], w[:D // 2]])
    sgn = np.concatenate([-np.ones(D // 2), np.ones(D // 2)])
    A = (cos * w[None, :]).astype(np.float32)
    Bt = (sin * w_rot[None, :] * sgn[None, :]).astype(np.float32)
    if np.any(b != 0):
        raise NotImplementedError("nonzero qk-norm bias not supported")
    return A, Bt


def kernel(**inputs):
    from ml_dtypes import bfloat16

    x = np.asarray(inputs["q"], dtype=np.float32)
    Wq = np.asarray(inputs["Wq"], dtype=np.float32)
    Wk = np.asarray(inputs["Wk"], dtype=np.float32)
    Wv = np.asarray(inputs["Wv"], dtype=np.float32)
    Wo = np.asarray(inputs["Wo"], dtype=np.float32)
    bo = np.asarray(inputs["bo"], dtype=np.float32)
    assert not np.any(bo != 0), "nonzero output bias not supported"

    Aq, Bq = _rope_tables(np.asarray(inputs["qn_w"], np.float32),
                          np.asarray(inputs["qn_b"], np.float32))
    Ak, Bk = _rope_tables(np.asarray(inputs["kn_w"], np.float32),
                          np.asarray(inputs["kn_b"], np.float32))
    WoT = np.ascontiguousarray(Wo.T)                          # [C(c'), C(o)]

    def _tbl(a):   # [L, D] -> [P, NT, D] (partition-major, contiguous DMA)
        return np.ascontiguousarray(
            a.reshape(NT, P, D).transpose(1, 0, 2)).astype(bfloat16)
    Aqr, Bqr, Akr, Bkr = _tbl(Aq), _tbl(Bq), _tbl(Ak), _tbl(Bk)

    if "nc" not in _NC_CACHE:
        _NC_CACHE["nc"] = _build_nc()
    nc = _NC_CACHE["nc"]

    in_maps = []
    for c in range(8):
        b_, g = c // 2, c % 2
        sl = slice(g * CG, (g + 1) * CG)
        in_maps.append({
            "xT": np.ascontiguousarray(
                x[b_].T.reshape(NCK, P, NT, P).transpose(1, 2, 0, 3)).astype(bfloat16),
            "wqT": np.ascontiguousarray(Wq[sl, :].T).astype(bfloat16),
            "wkT": np.ascontiguousarray(Wk[sl, :].T).astype(bfloat16),
            "wvT": np.ascontiguousarray(Wv[sl, :].T).astype(bfloat16),
            # [pair, 2*D rows (= the pair's context channels), C]
            "woT": np.ascontiguousarray(
                WoT[sl, :].reshape(NPR, P, C)).astype(bfloat16),
            "aq": Aqr, "bq": Bqr, "ak": Akr, "bk": Bkr,
        })

    res = run_bass_kernel_spmd(nc, in_maps, core_ids=list(range(8)))
    # each core wrote its full [L, C] o_proj partial; unshard = sum the two
    # head-group partials per batch
    out = np.empty((B, L, C), dtype=np.float32)
    for b_ in range(B):
        out[b_] = res.results[2 * b_]["out"] + res.results[2 * b_ + 1]["out"]
    return out


# revision 29
# speedup vs baseline: 1.1750x; 1.1750x over previous
"""MultiHeadAttention (qk-LayerNorm + RoPE) Trainium2 kernel, 8 NeuronCores.

Sharding: batch (4) x head-group (2x8 heads). Core c handles batch c//2,
heads 8*(c%2) .. 8*(c%2)+7. Each core computes QKV projections for its
batch restricted to its head group, per-head LayerNorm + rotary embedding,
attention, and a partial output projection over its 512 context channels.
The two partial o_proj results per batch are summed on the host (the
"unshard" step), which keeps the device program collective-free: no NEFF
entry barrier, no ReduceScatter tail.

Dataflow per core (all matmul operands bf16, PSUM accumulation fp32):
  Phase 1: per 128-token tile: QKV projections (bf16, x and per-ck weight
    tiles DMA'd in interleaved order so matmuls start early), LayerNorm
    stats via per-head reductions (merged across two token tiles),
    LN+rope applied in bf16, q/k transposed to [d, t] layout via PE
    transposes (two heads per 128x128 transpose).
  Phase 2: per head pair (row groups 0:64 / 64:128 of the packed q/k
    tiles): scores for both heads concurrently (distinct PE row groups),
    one exp ACT op per j covering both heads [128, 2048], ctx accumulated
    per 512-token half into 1-bank PSUM tiles (m=0 interleaved into the
    j-loop, m=1 as a dense burst over the retained exp tiles) with a ones
    column appended to v so the softmax denominator falls out of the same
    matmul. Normalization per half: DVE copy of the denominator row to
    partition 0, gpsimd partition_broadcast, reciprocal_approx_fast, one
    multiply per head; the odd head is moved to partitions 64:127 with a
    cross-quadrant stream_shuffle so o_proj runs K=128 per head pair.
  Phase 3: o_proj per token tile (4 accumulating K=128 matmuls, weights
    reused across the two 512-column halves), fp32 partial DMA'd to DRAM.
"""
import sys

for _p in ("/opt/trn_rl_repo", "/root/.axon_site", "/root/.axon_site/_ro/trn_rl_repo",
           "/root/.axon_site/_ro/pypackages"):
    if _p not in sys.path:
        sys.path.append(_p)

import numpy as np

import concourse.bass as bass
import concourse.tile as tile
from concourse import bacc, mybir
from concourse.bass_utils import run_bass_kernel_spmd
from concourse.masks import make_identity

F32 = mybir.dt.float32
F32R = mybir.dt.float32r
BF16 = mybir.dt.bfloat16
P = 128
B, L, C, H, D = 4, 1024, 1024, 16, 64
HC = 8          # heads per core
NPR = HC // 2   # head pairs per core
CG = HC * D     # 512 context channels per core
NT = L // P     # 8 token tiles
NCK = C // P    # 8 contraction tiles
THETA = 50000.0
EPS = 1e-5

_NC_CACHE = {}
# dummy keep-warm matmul counts (fill PE idle so the HAM clock gate stays
# at K=8/8; targets are PSUM slivers cleared by the next start=True group)
WARM1, WARM2, WARM2E, WARM3, WARM3PRE = 0, 0, 0, 0, 0


def _build_nc():
    nc = bacc.Bacc("TRN2", target_bir_lowering=False, debug=False, num_devices=8)

    xT_d = nc.dram_tensor("xT", [P, NT, NCK, P], BF16, kind="ExternalInput")
    wqT_d = nc.dram_tensor("wqT", [C, CG], BF16, kind="ExternalInput")
    wkT_d = nc.dram_tensor("wkT", [C, CG], BF16, kind="ExternalInput")
    wvT_d = nc.dram_tensor("wvT", [C, CG], BF16, kind="ExternalInput")
    woT_d = nc.dram_tensor("woT", [NPR, P, C], BF16, kind="ExternalInput")
    aq_d = nc.dram_tensor("aq", [P, NT, D], BF16, kind="ExternalInput")
    bq_d = nc.dram_tensor("bq", [P, NT, D], BF16, kind="ExternalInput")
    ak_d = nc.dram_tensor("ak", [P, NT, D], BF16, kind="ExternalInput")
    bk_d = nc.dram_tensor("bk", [P, NT, D], BF16, kind="ExternalInput")
    out_d = nc.dram_tensor("out", [L, C], F32, kind="ExternalOutput")

    with tile.TileContext(nc) as tc:
        with (
            tc.tile_pool(name="const", bufs=1) as constp,
            tc.tile_pool(name="w", bufs=1) as wpool,
            tc.tile_pool(name="big", bufs=1) as bigp,
            tc.tile_pool(name="scr", bufs=2) as scrp,
            tc.tile_pool(name="rope", bufs=2) as ropep,
            tc.tile_pool(name="stat", bufs=2) as statp,
            tc.tile_pool(name="exp", bufs=1) as expp,
            tc.tile_pool(name="den", bufs=2) as denp,
            tc.tile_pool(name="fin", bufs=2) as finp,
        ):
            ident = constp.tile([P, P], BF16)
            make_identity(nc, ident)
            eps_t = constp.tile([P, 1], F32)
            nc.vector.memset(eps_t[:], EPS)

            a2_t = constp.tile([P, 2, NT, D], BF16)
            b2_t = constp.tile([P, 2, NT, D], BF16)

            # x resident in SBUF, tile-major. DMA order: x tile 0, all wq,
            # all wk, x tile 1, all wv, x tiles 2..7 — so tile 0's q stats
            # (the head of the DVE pipeline) are ready after ~1.3MB of
            # traffic instead of the full 5MB
            xt_all = bigp.tile([P, NT, NCK, P], BF16)
            wq_t, wk_t, wv_t = [], [], []

            def _w_dmas(lst, nm, d_):
                for ck in range(NCK):
                    t_ = wpool.tile([P, CG], BF16, tag=f"{nm}{ck}", name=f"{nm}{ck}")
                    nc.sync.dma_start(
                        t_[:],
                        d_.ap().rearrange("(k p) o -> p k o", p=P)[:, ck, :])
                    lst.append(t_)

            nc.sync.dma_start(xt_all[:, 0], xT_d.ap()[:, 0])
            _w_dmas(wq_t, "wq", wqT_d)
            _w_dmas(wk_t, "wk", wkT_d)
            nc.sync.dma_start(xt_all[:, 1], xT_d.ap()[:, 1])
            _w_dmas(wv_t, "wv", wvT_d)
            for ti in range(2, NT):
                nc.sync.dma_start(xt_all[:, ti], xT_d.ap()[:, ti])

            nc.sync.dma_start(a2_t[:, 0, :, :], aq_d.ap())
            nc.sync.dma_start(a2_t[:, 1, :, :], ak_d.ap())
            nc.sync.dma_start(b2_t[:, 0, :, :], bq_d.ap())
            nc.sync.dma_start(b2_t[:, 1, :, :], bk_d.ap())

            # v with a ones column appended per head: [s_tile, j, head, 65]
            v_sb = bigp.tile([P, NT, HC, D + 1], BF16)
            nc.vector.memset(
                v_sb[:, :, :, D:D + 1].rearrange("p t h o -> p (t h) o"), 1.0)
            def warm(n, target):
                for _ in range(n):
                    nc.tensor.matmul(target[0:16, 0:16], xt_all[:, 0, 0, 16:32],
                                     xt_all[:, 0, 0, 0:16], start=True, stop=True)


            qT_pack = bigp.tile([P, NPR, L], BF16)
            kT_pack = bigp.tile([P, NPR, L], BF16)
            # ctx packed two heads per 128 partitions: [128, pair, L]
            ctxT2 = bigp.tile([P, NPR, L], BF16)

            # ---------------- Phase 1: QKV + LN + RoPE + transpose ----------
            # processed two token tiles per group: the QKV matmuls and the
            # PSUM-reading ops (reduces, squares, t1) run per tile, the rest
            # of the LN/rope arithmetic runs as merged [P, 2, 2, HC, *] DVE
            # ops to amortize per-op overhead and pipeline drains
            with tc.tile_pool(name="ps1", bufs=2, space="PSUM") as ps1, \
                 tc.tile_pool(name="pst", bufs=2, space="PSUM") as pst:
                for g in range(NT // 2):
                    stats = statp.tile([P, 2, 4, HC], F32, tag="stats")
                    qk_sb = scrp.tile([P, 2, 2, HC, D], BF16, tag="qk_sb")
                    t1 = qk_sb  # LN-apply runs in place on the bf16 copy
                    psqks = []
                    for s in range(2):
                        ti = 2 * g + s
                        psq = ps1.tile([P, CG], F32, tag="psq", name="psq")
                        psk = ps1.tile([P, CG], F32, tag="psk", name="psk")
                        psv = ps1.tile([P, CG], F32, tag="psv", name="psv")
                        psqks.append((psq, psk))
                        for ps_, w_ in ((psq, wq_t), (psk, wk_t), (psv, wv_t)):
                            for ck in range(NCK):
                                nc.tensor.matmul(
                                    ps_[:], xt_all[:, ti, ck, :], w_[ck][:],
                                    start=(ck == 0), stop=(ck == NCK - 1))

                        # v straight to SBUF (bf16); ACT to keep DVE free
                        nc.scalar.copy(
                            v_sb[:, ti, :, 0:D],
                            psv[:].rearrange("p (h d) -> p h d", d=D))

                        # sums / sums of squares per (token, q/k, head);
                        # q/k also copied to bf16 SBUF (ACT) so the PSUM
                        # banks free early and the rope chain runs at the
                        # 2x bf16 DVE rate
                        for i, ps_ in enumerate((psq, psk)):
                            nc.vector.reduce_sum(
                                stats[:, s, 2 * i, :],
                                ps_[:].rearrange("p (h d) -> p h d", d=D),
                                axis=mybir.AxisListType.X)
                            nc.scalar.copy(qk_sb[:, s, i], ps_[:].rearrange(
                                "p (h d) -> p h d", d=D))
                            sq = scrp.tile([P, CG], F32, tag="sq")
                            nc.scalar.square(sq[:], ps_[:])
                            nc.vector.reduce_sum(
                                stats[:, s, 2 * i + 1, :],
                                sq[:].rearrange("p (h d) -> p h d", d=D),
                                axis=mybir.AxisListType.X)
                    mu2 = statp.tile([P, 2, 4, HC], F32, tag="mu2")
                    nc.vector.tensor_scalar_mul(mu2[:], stats[:], 1.0 / D)
                    var = statp.tile([P, 2, 2, HC], F32, tag="var")
                    nc.vector.tensor_mul(var[:], mu2[:, :, 0::2, :], mu2[:, :, 0::2, :])
                    nc.vector.tensor_sub(var[:], mu2[:, :, 1::2, :], var[:])
                    std = statp.tile([P, 2, 2, HC], F32, tag="std")
                    nc.scalar.activation(std[:], var[:],
                                         mybir.ActivationFunctionType.Sqrt,
                                         bias=eps_t[:])
                    inv = statp.tile([P, 2, 2, HC], F32, tag="inv")
                    nc.vector.reciprocal(inv[:], std[:])
                    invh = statp.tile([P, 2, 2, HC], BF16, tag="invh")
                    nc.vector.tensor_copy(invh[:], inv[:])
                    shifth = statp.tile([P, 2, 2, HC], BF16, tag="shifth")
                    nc.vector.tensor_mul(shifth[:], mu2[:, :, 0::2, :], inv[:])

                    h_ = D // 2
                    for s in range(2):
                        ti = 2 * g + s
                        inv_b = invh[:, s].rearrange("p i h -> p i h ()").to_broadcast(
                            (P, 2, HC, D))
                        sh_b = shifth[:, s].rearrange("p i h -> p i h ()").to_broadcast(
                            (P, 2, HC, D))
                        a_b = a2_t[:, :, ti, :].rearrange(
                            "p i d -> p i () d").to_broadcast((P, 2, HC, D))
                        nc.vector.tensor_mul(t1[:, s], t1[:, s], inv_b)
                        nc.vector.tensor_sub(t1[:, s], t1[:, s], sh_b)
                        rope = ropep.tile([P, 2, HC, D], BF16, tag=f"rope{s}")
                        nc.vector.tensor_mul(rope[:], t1[:, s], a_b)
                        r2 = scrp.tile([P, 2, HC, D], BF16, tag=f"r2{s}")
                        nc.vector.tensor_mul(
                            r2[:, :, :, 0:h_], t1[:, s, :, :, h_:D],
                            b2_t[:, :, ti, 0:h_].rearrange(
                                "p i d -> p i () d").to_broadcast((P, 2, HC, h_)))
                        nc.vector.tensor_mul(
                            r2[:, :, :, h_:D], t1[:, s, :, :, 0:h_],
                            b2_t[:, :, ti, h_:D].rearrange(
                                "p i d -> p i () d").to_broadcast((P, 2, HC, h_)))
                        nc.vector.tensor_add(rope[:], rope[:], r2[:])
                        for i, dstpack in ((0, qT_pack), (1, kT_pack)):
                            for pr in range(NPR):
                                ps_t = pst.tile([P, P], BF16)
                                nc.tensor.transpose(
                                    ps_t[:],
                                    rope[:, i, 2 * pr:2 * pr + 2, :].rearrange(
                                        "p h d -> p (h d)"),
                                    ident[:])
                                nc.scalar.copy(dstpack[:, pr, bass.ts(ti, P)], ps_t[:])
                    warm(WARM1, psqks[0][0])

            # o_proj weights early: reuses the per-ck wq slots (dead after
            # phase 1); packed per head pair [128, C] to match ctxT2
            wo_l = []
            for pr in range(NPR):
                wo_p = wpool.tile([P, C], BF16, tag=f"wq{pr}", name=f"wo{pr}")
                nc.sync.dma_start(wo_p[:], woT_d.ap()[pr, :, :])
                wo_l.append(wo_p)

            # ---------------- Phase 2: attention per head pair --------------
            # ctx accumulates per 512-token half into 1-bank PSUM tiles:
            # m=0 interleaved into the scores/exp j-loop, m=1 as a dense
            # matmul burst afterwards (all exp tiles are kept in SBUF).
            # Each half normalizes independently, so no pair-boundary
            # barrier on PSUM and the PE never idles long enough for the
            # HAM clock gate to re-throttle.
            with tc.tile_pool(name="pss", bufs=1, space="PSUM") as pssp, \
                 tc.tile_pool(name="psc", bufs=1, space="PSUM") as pscp:
                shuffle_ident = list(range(32))

                def normalize(pr, head, m, psc_):
                    den = denp.tile([1, 512], F32, tag=f"den{head}{m}")
                    nc.vector.tensor_copy(den[0:1, :], psc_[D:D + 1, :])
                    rbr = denp.tile([D, 512], F32, tag=f"rbr{head}{m}")
                    nc.gpsimd.partition_broadcast(rbr[:], den[0:1, :])
                    rb = denp.tile([D, 512], F32, tag=f"rb{head}{m}")
                    nc.vector.reciprocal_approx_fast(rb[:], rbr[:])
                    if head == 0:
                        nc.vector.tensor_mul(
                            ctxT2[0:D, pr, bass.ts(m, 512)], psc_[0:D, :], rb[:])
                    else:
                        tmpB = denp.tile([D, 512], BF16, tag=f"tmpB{m}")
                        nc.vector.tensor_mul(tmpB[:], psc_[0:D, :], rb[:])
                        nc.vector.stream_shuffle(
                            ctxT2[D:2 * D, pr, bass.ts(m, 512)], tmpB[:],
                            shuffle_ident)

                for pr in range(NPR):
                    hA, hB = 2 * pr, 2 * pr + 1
                    psc0 = [pscp.tile([D + 1, 512], F32, tag=f"pc{h}0",
                                      name=f"pc{h}0") for h in range(2)]
                    psc1 = [pscp.tile([D + 1, 512], F32, tag=f"pc{h}1",
                                      name=f"pc{h}1") for h in range(2)]
                    exps = []
                    for j in range(NT):
                        pss = pssp.tile([P, 2, 2, 512], F32, tag="pss")
                        for m in range(2):
                            for half in range(2):
                                nc.tensor.matmul(
                                    pss[:, half, m, :],
                                    kT_pack[half * D:(half + 1) * D, pr, bass.ts(j, P)],
                                    qT_pack[half * D:(half + 1) * D, pr, bass.ts(m, 512)],
                                    start=True, stop=True)
                        expAB = expp.tile([P, 2, 2, 512], BF16, tag=f"expAB{j}",
                                          name=f"expAB{j}")
                        nc.scalar.activation(expAB[:], pss[:],
                                             mybir.ActivationFunctionType.Exp,
                                             scale=float(D) ** -0.5)
                        exps.append(expAB)
                        # ctx m=0 for the previous j, interleaved with scores
                        if j >= 1:
                            for head, h in ((0, hA), (1, hB)):
                                nc.tensor.matmul(
                                    psc0[head][:], v_sb[:, j - 1, h, :],
                                    exps[j - 1][:, head, 0, :],
                                    start=(j - 1 == 0), stop=(j - 1 == NT - 1))
                            warm(WARM2, psc1[0])
                    for head, h in ((0, hA), (1, hB)):
                        nc.tensor.matmul(
                            psc0[head][:], v_sb[:, NT - 1, h, :],
                            exps[NT - 1][:, head, 0, :],
                            start=False, stop=True)
                    for j in range(NT):
                        for head, h in ((0, hA), (1, hB)):
                            nc.tensor.matmul(
                                psc1[head][:], v_sb[:, j, h, :],
                                exps[j][:, head, 1, :],
                                start=(j == 0), stop=(j == NT - 1))
                    for head in range(2):
                        normalize(pr, head, 0, psc0[head])
                    for head in range(2):
                        normalize(pr, head, 1, psc1[head])

            # ---------------- Phase 3: output projection --------------------
            with tc.tile_pool(name="pso", bufs=2, space="PSUM") as psop:
                for ti in range(NT):
                    pso = psop.tile([P, C], F32, name="pso")
                    for pr in range(NPR):
                        for m in range(2):
                            nc.tensor.matmul(
                                pso[:, bass.ts(m, 512)],
                                ctxT2[:, pr, bass.ts(ti, P)],
                                wo_l[pr][:, bass.ts(m, 512)],
                                start=(pr == 0), stop=(pr == NPR - 1))
                    out_sb = finp.tile([P, C], F32, tag="out", name="out_sb")
                    if ti % 2 == 0:
                        nc.scalar.copy(out_sb[:], pso[:])
                    else:
                        nc.vector.tensor_copy(out_sb[:], pso[:])
                    nc.sync.dma_start(out_d.ap()[bass.ts(ti, P), :], out_sb[:])

    nc.compile()
    return nc


def _rope_tables(w, b):
    """A[t,d], B[t,d] with the rotate-half sign folded into B."""
    inv_freq = 1.0 / THETA ** (np.arange(0, D, 2, dtype=np.float64) / D)
    freqs = np.arange(L, dtype=np.float64)[:, None] * inv_freq[None, :]
    freqs = np.concatenate([freqs, freqs], axis=1)           # [L, D]
    cos, sin = np.cos(freqs), np.sin(freqs)
    w = w.astype(np.float64)
    w_rot = np.concatenate([w[D // 2:], w[:D // 2]])
    sgn = np.concatenate([-np.ones(D // 2), np.ones(D // 2)])
    A = (cos * w[None, :]).astype(np.float32)
    Bt = (sin * w_rot[None, :] * sgn[None, :]).astype(np.float32)
    if np.any(b != 0):
        raise NotImplementedError("nonzero qk-norm bias not supported")
    return A, Bt


def kernel(**inputs):
    from ml_dtypes import bfloat16

    x = np.asarray(inputs["q"], dtype=np.float32)
    Wq = np.asarray(inputs["Wq"], dtype=np.float32)
    Wk = np.asarray(inputs["Wk"], dtype=np.float32)
    Wv = np.asarray(inputs["Wv"], dtype=np.float32)
    Wo = np.asarray(inputs["Wo"], dtype=np.float32)
    bo = np.asarray(inputs["bo"], dtype=np.float32)
    assert not np.any(bo != 0), "nonzero output bias not supported"

    Aq, Bq = _rope_tables(np.asarray(inputs["qn_w"], np.float32),
                          np.asarray(inputs["qn_b"], np.float32))
    Ak, Bk = _rope_tables(np.asarray(inputs["kn_w"], np.float32),
                          np.asarray(inputs["kn_b"], np.float32))
    WoT = np.ascontiguousarray(Wo.T)                          # [C(c'), C(o)]

    def _tbl(a):   # [L, D] -> [P, NT, D] (partition-major, contiguous DMA)
        return np.ascontiguousarray(
            a.reshape(NT, P, D).transpose(1, 0, 2)).astype(bfloat16)
    Aqr, Bqr, Akr, Bkr = _tbl(Aq), _tbl(Bq), _tbl(Ak), _tbl(Bk)

    if "nc" not in _NC_CACHE:
        _NC_CACHE["nc"] = _build_nc()
    nc = _NC_CACHE["nc"]

    in_maps = []
    for c in range(8):
        b_, g = c // 2, c % 2
        sl = slice(g * CG, (g + 1) * CG)
        in_maps.append({
            "xT": np.ascontiguousarray(
                x[b_].T.reshape(NCK, P, NT, P).transpose(1, 2, 0, 3)).astype(bfloat16),
            "wqT": np.ascontiguousarray(Wq[sl, :].T).astype(bfloat16),
            "wkT": np.ascontiguousarray(Wk[sl, :].T).astype(bfloat16),
            "wvT": np.ascontiguousarray(Wv[sl, :].T).astype(bfloat16),
            # [pair, 2*D rows (= the pair's context channels), C]
            "woT": np.ascontiguousarray(
                WoT[sl, :].reshape(NPR, P, C)).astype(bfloat16),
            "aq": Aqr, "bq": Bqr, "ak": Akr, "bk": Bkr,
        })

    res = run_bass_kernel_spmd(nc, in_maps, core_ids=list(range(8)))
    # each core wrote its full [L, C] o_proj partial; unshard = sum the two
    # head-group partials per batch
    out = np.empty((B, L, C), dtype=np.float32)
    for b_ in range(B):
        out[b_] = res.results[2 * b_]["out"] + res.results[2 * b_ + 1]["out"]
    return out


# revision 30
# speedup vs baseline: 1.1805x; 1.0047x over previous
"""MultiHeadAttention (qk-LayerNorm + RoPE) Trainium2 kernel, 8 NeuronCores.

Sharding: batch (4) x head-group (2x8 heads). Core c handles batch c//2,
heads 8*(c%2) .. 8*(c%2)+7. Each core computes QKV projections for its
batch restricted to its head group, per-head LayerNorm + rotary embedding,
attention, and a partial output projection over its 512 context channels.
The two partial o_proj results per batch are summed on the host (the
"unshard" step), which keeps the device program collective-free: no NEFF
entry barrier, no ReduceScatter tail.

Dataflow per core (all matmul operands bf16, PSUM accumulation fp32):
  Phase 1: per 128-token tile: QKV projections (bf16, x and per-ck weight
    tiles DMA'd in interleaved order so matmuls start early), LayerNorm
    stats via per-head reductions (merged across two token tiles),
    LN+rope applied in bf16, q/k transposed to [d, t] layout via PE
    transposes (two heads per 128x128 transpose).
  Phase 2: per head pair (row groups 0:64 / 64:128 of the packed q/k
    tiles): scores for both heads concurrently (distinct PE row groups),
    one exp ACT op per j covering both heads [128, 2048], ctx accumulated
    per 512-token half into 1-bank PSUM tiles (m=0 interleaved into the
    j-loop, m=1 as a dense burst over the retained exp tiles) with a ones
    column appended to v so the softmax denominator falls out of the same
    matmul. Normalization per half: DVE copy of the denominator row to
    partition 0, gpsimd partition_broadcast, reciprocal_approx_fast, one
    multiply per head; the odd head is moved to partitions 64:127 with a
    cross-quadrant stream_shuffle so o_proj runs K=128 per head pair.
  Phase 3: o_proj per token tile (4 accumulating K=128 matmuls, weights
    reused across the two 512-column halves), fp32 partial DMA'd to DRAM.
"""
import sys

for _p in ("/opt/trn_rl_repo", "/root/.axon_site", "/root/.axon_site/_ro/trn_rl_repo",
           "/root/.axon_site/_ro/pypackages"):
    if _p not in sys.path:
        sys.path.append(_p)

import numpy as np

import concourse.bass as bass
import concourse.tile as tile
from concourse import bacc, mybir
from concourse.bass_utils import run_bass_kernel_spmd
from concourse.masks import make_identity

F32 = mybir.dt.float32
F32R = mybir.dt.float32r
BF16 = mybir.dt.bfloat16
P = 128
B, L, C, H, D = 4, 1024, 1024, 16, 64
HC = 8          # heads per core
NPR = HC // 2   # head pairs per core
CG = HC * D     # 512 context channels per core
NT = L // P     # 8 token tiles
NCK = C // P    # 8 contraction tiles
THETA = 50000.0
EPS = 1e-5

_NC_CACHE = {}
# dummy keep-warm matmul counts (fill PE idle so the HAM clock gate stays
# at K=8/8; targets are PSUM slivers cleared by the next start=True group)
WARM1, WARM2, WARM2E, WARM3, WARM3PRE = 0, 0, 0, 0, 0


def _build_nc():
    nc = bacc.Bacc("TRN2", target_bir_lowering=False, debug=False, num_devices=8)

    xT_d = nc.dram_tensor("xT", [P, NT, NCK, P], BF16, kind="ExternalInput")
    wqT_d = nc.dram_tensor("wqT", [C, CG], BF16, kind="ExternalInput")
    wkT_d = nc.dram_tensor("wkT", [C, CG], BF16, kind="ExternalInput")
    wvT_d = nc.dram_tensor("wvT", [C, CG], BF16, kind="ExternalInput")
    woT_d = nc.dram_tensor("woT", [NPR, P, C], BF16, kind="ExternalInput")
    aq_d = nc.dram_tensor("aq", [P, NT, D], BF16, kind="ExternalInput")
    bq_d = nc.dram_tensor("bq", [P, NT, D], BF16, kind="ExternalInput")
    ak_d = nc.dram_tensor("ak", [P, NT, D], BF16, kind="ExternalInput")
    bk_d = nc.dram_tensor("bk", [P, NT, D], BF16, kind="ExternalInput")
    out_d = nc.dram_tensor("out", [L, C], F32, kind="ExternalOutput")

    with tile.TileContext(nc) as tc:
        with (
            tc.tile_pool(name="const", bufs=1) as constp,
            tc.tile_pool(name="w", bufs=1) as wpool,
            tc.tile_pool(name="big", bufs=1) as bigp,
            tc.tile_pool(name="scr", bufs=2) as scrp,
            tc.tile_pool(name="rope", bufs=2) as ropep,
            tc.tile_pool(name="stat", bufs=2) as statp,
            tc.tile_pool(name="exp", bufs=1) as expp,
            tc.tile_pool(name="den", bufs=2) as denp,
            tc.tile_pool(name="fin", bufs=2) as finp,
        ):
            ident = constp.tile([P, P], BF16)
            make_identity(nc, ident)
            eps_t = constp.tile([P, 1], F32)
            nc.vector.memset(eps_t[:], EPS)

            a2_t = constp.tile([P, 2, NT, D], BF16)
            b2_t = constp.tile([P, 2, NT, D], BF16)

            # x resident in SBUF, tile-major. DMA order: x tile 0, all wq,
            # all wk, x tile 1, all wv, x tiles 2..7 — so tile 0's q stats
            # (the head of the DVE pipeline) are ready after ~1.3MB of
            # traffic instead of the full 5MB
            xt_all = bigp.tile([P, NT, NCK, P], BF16)
            wq_t, wk_t, wv_t = [], [], []

            def _w_dmas(lst, nm, d_):
                for ck in range(NCK):
                    t_ = wpool.tile([P, CG], BF16, tag=f"{nm}{ck}", name=f"{nm}{ck}")
                    nc.sync.dma_start(
                        t_[:],
                        d_.ap().rearrange("(k p) o -> p k o", p=P)[:, ck, :])
                    lst.append(t_)

            nc.sync.dma_start(xt_all[:, 0], xT_d.ap()[:, 0])
            _w_dmas(wq_t, "wq", wqT_d)
            _w_dmas(wk_t, "wk", wkT_d)
            nc.sync.dma_start(xt_all[:, 1], xT_d.ap()[:, 1])
            _w_dmas(wv_t, "wv", wvT_d)
            for ti in range(2, NT):
                nc.sync.dma_start(xt_all[:, ti], xT_d.ap()[:, ti])

            nc.sync.dma_start(a2_t[:, 0, :, :], aq_d.ap())
            nc.sync.dma_start(a2_t[:, 1, :, :], ak_d.ap())
            nc.sync.dma_start(b2_t[:, 0, :, :], bq_d.ap())
            nc.sync.dma_start(b2_t[:, 1, :, :], bk_d.ap())

            # v with a ones column appended per head: [s_tile, j, head, 65]
            v_sb = bigp.tile([P, NT, HC, D + 1], BF16)
            nc.vector.memset(
                v_sb[:, :, :, D:D + 1].rearrange("p t h o -> p (t h) o"), 1.0)
            def warm(n, target):
                for _ in range(n):
                    nc.tensor.matmul(target[0:16, 0:16], xt_all[:, 0, 0, 16:32],
                                     xt_all[:, 0, 0, 0:16], start=True, stop=True)


            qT_pack = bigp.tile([P, NPR, L], BF16)
            kT_pack = bigp.tile([P, NPR, L], BF16)
            # ctx packed two heads per 128 partitions: [128, pair, L]
            ctxT2 = bigp.tile([P, NPR, L], BF16)

            # ---------------- Phase 1: QKV + LN + RoPE + transpose ----------
            # processed two token tiles per group: the QKV matmuls and the
            # PSUM-reading ops (reduces, squares, t1) run per tile, the rest
            # of the LN/rope arithmetic runs as merged [P, 2, 2, HC, *] DVE
            # ops to amortize per-op overhead and pipeline drains
            with tc.tile_pool(name="ps1", bufs=2, space="PSUM") as ps1, \
                 tc.tile_pool(name="pst", bufs=2, space="PSUM") as pst:
                for g in range(NT // 2):
                    stats = statp.tile([P, 2, 4, HC], F32, tag="stats")
                    qk_sb = scrp.tile([P, 2, 2, HC, D], BF16, tag="qk_sb")
                    t1 = qk_sb  # LN-apply runs in place on the bf16 copy
                    psqks = []
                    for s in range(2):
                        ti = 2 * g + s
                        psq = ps1.tile([P, CG], F32, tag="psq", name="psq")
                        psk = ps1.tile([P, CG], F32, tag="psk", name="psk")
                        psv = ps1.tile([P, CG], F32, tag="psv", name="psv")
                        psqks.append((psq, psk))
                        for ps_, w_ in ((psq, wq_t), (psk, wk_t), (psv, wv_t)):
                            for ck in range(NCK):
                                nc.tensor.matmul(
                                    ps_[:], xt_all[:, ti, ck, :], w_[ck][:],
                                    start=(ck == 0), stop=(ck == NCK - 1))

                        # v straight to SBUF (bf16); ACT to keep DVE free
                        nc.scalar.copy(
                            v_sb[:, ti, :, 0:D],
                            psv[:].rearrange("p (h d) -> p h d", d=D))

                        # sums / sums of squares per (token, q/k, head);
                        # q/k also copied to bf16 SBUF (ACT) so the PSUM
                        # banks free early and the rope chain runs at the
                        # 2x bf16 DVE rate
                        for i, ps_ in enumerate((psq, psk)):
                            nc.vector.reduce_sum(
                                stats[:, s, 2 * i, :],
                                ps_[:].rearrange("p (h d) -> p h d", d=D),
                                axis=mybir.AxisListType.X)
                            nc.scalar.copy(qk_sb[:, s, i], ps_[:].rearrange(
                                "p (h d) -> p h d", d=D))
                            sq = scrp.tile([P, CG], F32, tag="sq")
                            nc.scalar.square(sq[:], ps_[:])
                            nc.vector.reduce_sum(
                                stats[:, s, 2 * i + 1, :],
                                sq[:].rearrange("p (h d) -> p h d", d=D),
                                axis=mybir.AxisListType.X)
                    mu2 = statp.tile([P, 2, 4, HC], F32, tag="mu2")
                    nc.vector.tensor_scalar_mul(mu2[:], stats[:], 1.0 / D)
                    var = statp.tile([P, 2, 2, HC], F32, tag="var")
                    nc.vector.tensor_mul(var[:], mu2[:, :, 0::2, :], mu2[:, :, 0::2, :])
                    nc.vector.tensor_sub(var[:], mu2[:, :, 1::2, :], var[:])
                    std = statp.tile([P, 2, 2, HC], F32, tag="std")
                    nc.scalar.activation(std[:], var[:],
                                         mybir.ActivationFunctionType.Sqrt,
                                         bias=eps_t[:])
                    inv = statp.tile([P, 2, 2, HC], F32, tag="inv")
                    nc.vector.reciprocal(inv[:], std[:])
                    invh = statp.tile([P, 2, 2, HC], BF16, tag="invh")
                    nc.vector.tensor_copy(invh[:], inv[:])
                    shifth = statp.tile([P, 2, 2, HC], BF16, tag="shifth")
                    nc.vector.tensor_mul(shifth[:], mu2[:, :, 0::2, :], inv[:])

                    h_ = D // 2
                    for s in range(2):
                        ti = 2 * g + s
                        inv_b = invh[:, s].rearrange("p i h -> p i h ()").to_broadcast(
                            (P, 2, HC, D))
                        sh_b = shifth[:, s].rearrange("p i h -> p i h ()").to_broadcast(
                            (P, 2, HC, D))
                        a_b = a2_t[:, :, ti, :].rearrange(
                            "p i d -> p i () d").to_broadcast((P, 2, HC, D))
                        nc.vector.tensor_mul(t1[:, s], t1[:, s], inv_b)
                        nc.vector.tensor_sub(t1[:, s], t1[:, s], sh_b)
                        rope = ropep.tile([P, 2, HC, D], BF16, tag=f"rope{s}")
                        nc.vector.tensor_mul(rope[:], t1[:, s], a_b)
                        r2 = scrp.tile([P, 2, HC, D], BF16, tag=f"r2{s}")
                        nc.vector.tensor_mul(
                            r2[:, :, :, 0:h_], t1[:, s, :, :, h_:D],
                            b2_t[:, :, ti, 0:h_].rearrange(
                                "p i d -> p i () d").to_broadcast((P, 2, HC, h_)))
                        nc.vector.tensor_mul(
                            r2[:, :, :, h_:D], t1[:, s, :, :, 0:h_],
                            b2_t[:, :, ti, h_:D].rearrange(
                                "p i d -> p i () d").to_broadcast((P, 2, HC, h_)))
                        nc.vector.tensor_add(rope[:], rope[:], r2[:])
                        for i, dstpack in ((0, qT_pack), (1, kT_pack)):
                            for pr in range(NPR):
                                ps_t = pst.tile([P, P], BF16)
                                nc.tensor.transpose(
                                    ps_t[:],
                                    rope[:, i, 2 * pr:2 * pr + 2, :].rearrange(
                                        "p h d -> p (h d)"),
                                    ident[:])
                                nc.scalar.copy(dstpack[:, pr, bass.ts(ti, P)], ps_t[:])
                    warm(WARM1, psqks[0][0])

            # o_proj weights early: reuses the per-ck wq slots (dead after
            # phase 1); packed per head pair [128, C] to match ctxT2
            wo_l = []
            for pr in range(NPR):
                wo_p = wpool.tile([P, C], BF16, tag=f"wq{pr}", name=f"wo{pr}")
                nc.sync.dma_start(wo_p[:], woT_d.ap()[pr, :, :])
                wo_l.append(wo_p)

            # ---------------- Phase 2: attention per head pair --------------
            # ctx accumulates per 512-token half into 1-bank PSUM tiles:
            # m=0 interleaved into the scores/exp j-loop, m=1 as a dense
            # matmul burst afterwards (all exp tiles are kept in SBUF).
            # Each half normalizes independently, so no pair-boundary
            # barrier on PSUM and the PE never idles long enough for the
            # HAM clock gate to re-throttle.
            with tc.tile_pool(name="pss", bufs=1, space="PSUM") as pssp, \
                 tc.tile_pool(name="psc", bufs=1, space="PSUM") as pscp:
                shuffle_ident = list(range(32))

                def normalize(pr, head, m, psc_):
                    den = denp.tile([1, 512], F32, tag=f"den{head}{m}")
                    nc.vector.tensor_copy(den[0:1, :], psc_[D:D + 1, :])
                    rbr = denp.tile([D, 512], F32, tag=f"rbr{head}{m}")
                    nc.gpsimd.partition_broadcast(rbr[:], den[0:1, :])
                    rb = denp.tile([D, 512], F32, tag=f"rb{head}{m}")
                    nc.vector.reciprocal_approx_fast(rb[:], rbr[:])
                    if head == 0:
                        nc.vector.tensor_mul(
                            ctxT2[0:D, pr, bass.ts(m, 512)], psc_[0:D, :], rb[:])
                    else:
                        tmpB = denp.tile([D, 512], BF16, tag=f"tmpB{m}")
                        nc.vector.tensor_mul(tmpB[:], psc_[0:D, :], rb[:])
                        nc.vector.stream_shuffle(
                            ctxT2[D:2 * D, pr, bass.ts(m, 512)], tmpB[:],
                            shuffle_ident)

                for pr in range(NPR):
                    hA, hB = 2 * pr, 2 * pr + 1
                    psc0 = [pscp.tile([D + 1, 512], F32, tag=f"pc{h}0",
                                      name=f"pc{h}0") for h in range(2)]
                    psc1 = [pscp.tile([D + 1, 512], F32, tag=f"pc{h}1",
                                      name=f"pc{h}1") for h in range(2)]
                    exps = []
                    for j in range(NT):
                        pss = pssp.tile([P, 2, 2, 512], F32, tag="pss")
                        for m in range(2):
                            for half in range(2):
                                nc.tensor.matmul(
                                    pss[:, half, m, :],
                                    kT_pack[half * D:(half + 1) * D, pr, bass.ts(j, P)],
                                    qT_pack[half * D:(half + 1) * D, pr, bass.ts(m, 512)],
                                    start=True, stop=True)
                        expAB = expp.tile([P, 2, 2, 512], BF16, tag=f"expAB{j}",
                                          name=f"expAB{j}")
                        nc.scalar.activation(expAB[:], pss[:],
                                             mybir.ActivationFunctionType.Exp,
                                             scale=float(D) ** -0.5)
                        exps.append(expAB)
                        # ctx matmuls trail the scores/exp pipeline: m=0 at
                        # lag 2 (so the previous pair's normalize has freed
                        # the m0 accumulator before this hits the PE queue),
                        # m=1 at lag 4 (ditto, and it spreads the exp-tile
                        # WAR reads through the loop instead of a burst at
                        # the pair end that would stall the next pair's exps)
                        def ctx_mm(jc, m, psc_):
                            for head, h in ((0, hA), (1, hB)):
                                nc.tensor.matmul(
                                    psc_[head][:], v_sb[:, jc, h, :],
                                    exps[jc][:, head, m, :],
                                    start=(jc == 0), stop=(jc == NT - 1))
                        if j >= 2:
                            ctx_mm(j - 2, 0, psc0)
                        if j >= 4:
                            ctx_mm(j - 4, 1, psc1)
                    for jc in range(NT - 2, NT):
                        ctx_mm(jc, 0, psc0)
                    for jc in range(NT - 4, NT):
                        ctx_mm(jc, 1, psc1)
                    for head in range(2):
                        normalize(pr, head, 0, psc0[head])
                    for head in range(2):
                        normalize(pr, head, 1, psc1[head])

            # ---------------- Phase 3: output projection --------------------
            with tc.tile_pool(name="pso", bufs=2, space="PSUM") as psop:
                for ti in range(NT):
                    pso = psop.tile([P, C], F32, name="pso")
                    for pr in range(NPR):
                        for m in range(2):
                            nc.tensor.matmul(
                                pso[:, bass.ts(m, 512)],
                                ctxT2[:, pr, bass.ts(ti, P)],
                                wo_l[pr][:, bass.ts(m, 512)],
                                start=(pr == 0), stop=(pr == NPR - 1))
                    out_sb = finp.tile([P, C], F32, tag="out", name="out_sb")
                    if ti % 2 == 0:
                        nc.scalar.copy(out_sb[:], pso[:])
                    else:
                        nc.vector.tensor_copy(out_sb[:], pso[:])
                    nc.sync.dma_start(out_d.ap()[bass.ts(ti, P), :], out_sb[:])

    nc.compile()
    return nc


def _rope_tables(w, b):
    """A[t,d], B[t,d] with the rotate-half sign folded into B."""
    inv_freq = 1.0 / THETA ** (np.arange(0, D, 2, dtype=np.float64) / D)
    freqs = np.arange(L, dtype=np.float64)[:, None] * inv_freq[None, :]
    freqs = np.concatenate([freqs, freqs], axis=1)           # [L, D]
    cos, sin = np.cos(freqs), np.sin(freqs)
    w = w.astype(np.float64)
    w_rot = np.concatenate([w[D // 2:], w[:D // 2]])
    sgn = np.concatenate([-np.ones(D // 2), np.ones(D // 2)])
    A = (cos * w[None, :]).astype(np.float32)
    Bt = (sin * w_rot[None, :] * sgn[None, :]).astype(np.float32)
    if np.any(b != 0):
        raise NotImplementedError("nonzero qk-norm bias not supported")
    return A, Bt


def kernel(**inputs):
    from ml_dtypes import bfloat16

    x = np.asarray(inputs["q"], dtype=np.float32)
    Wq = np.asarray(inputs["Wq"], dtype=np.float32)
    Wk = np.asarray(inputs["Wk"], dtype=np.float32)
    Wv = np.asarray(inputs["Wv"], dtype=np.float32)
    Wo = np.asarray(inputs["Wo"], dtype=np.float32)
    bo = np.asarray(inputs["bo"], dtype=np.float32)
    assert not np.any(bo != 0), "nonzero output bias not supported"

    Aq, Bq = _rope_tables(np.asarray(inputs["qn_w"], np.float32),
                          np.asarray(inputs["qn_b"], np.float32))
    Ak, Bk = _rope_tables(np.asarray(inputs["kn_w"], np.float32),
                          np.asarray(inputs["kn_b"], np.float32))
    WoT = np.ascontiguousarray(Wo.T)                          # [C(c'), C(o)]

    def _tbl(a):   # [L, D] -> [P, NT, D] (partition-major, contiguous DMA)
        return np.ascontiguousarray(
            a.reshape(NT, P, D).transpose(1, 0, 2)).astype(bfloat16)
    Aqr, Bqr, Akr, Bkr = _tbl(Aq), _tbl(Bq), _tbl(Ak), _tbl(Bk)

    if "nc" not in _NC_CACHE:
        _NC_CACHE["nc"] = _build_nc()
    nc = _NC_CACHE["nc"]

    in_maps = []
    for c in range(8):
        b_, g = c // 2, c % 2
        sl = slice(g * CG, (g + 1) * CG)
        in_maps.append({
            "xT": np.ascontiguousarray(
                x[b_].T.reshape(NCK, P, NT, P).transpose(1, 2, 0, 3)).astype(bfloat16),
            "wqT": np.ascontiguousarray(Wq[sl, :].T).astype(bfloat16),
            "wkT": np.ascontiguousarray(Wk[sl, :].T).astype(bfloat16),
            "wvT": np.ascontiguousarray(Wv[sl, :].T).astype(bfloat16),
            # [pair, 2*D rows (= the pair's context channels), C]
            "woT": np.ascontiguousarray(
                WoT[sl, :].reshape(NPR, P, C)).astype(bfloat16),
            "aq": Aqr, "bq": Bqr, "ak": Akr, "bk": Bkr,
        })

    res = run_bass_kernel_spmd(nc, in_maps, core_ids=list(range(8)))
    # each core wrote its full [L, C] o_proj partial; unshard = sum the two
    # head-group partials per batch
    out = np.empty((B, L, C), dtype=np.float32)
    for b_ in range(B):
        out[b_] = res.results[2 * b_]["out"] + res.results[2 * b_ + 1]["out"]
    return out


# revision 31
# speedup vs baseline: 1.1826x; 1.0018x over previous
"""MultiHeadAttention (qk-LayerNorm + RoPE) Trainium2 kernel, 8 NeuronCores.

Sharding: batch (4) x head-group (2x8 heads). Core c handles batch c//2,
heads 8*(c%2) .. 8*(c%2)+7. Each core computes QKV projections for its
batch restricted to its head group, per-head LayerNorm + rotary embedding,
attention, and a partial output projection over its 512 context channels.
The two partial o_proj results per batch are summed on the host (the
"unshard" step), which keeps the device program collective-free: no NEFF
entry barrier, no ReduceScatter tail.

Dataflow per core (all matmul operands bf16, PSUM accumulation fp32):
  Phase 1: per 128-token tile: QKV projections (bf16, x and per-ck weight
    tiles DMA'd in interleaved order so matmuls start early), LayerNorm
    stats via per-head reductions (merged across two token tiles),
    LN+rope applied in bf16, q/k transposed to [d, t] layout via PE
    transposes (two heads per 128x128 transpose).
  Phase 2: per head pair (row groups 0:64 / 64:128 of the packed q/k
    tiles): scores for both heads concurrently (distinct PE row groups),
    one exp ACT op per j covering both heads [128, 2048], ctx accumulated
    per 512-token half into 1-bank PSUM tiles (m=0 interleaved into the
    j-loop, m=1 as a dense burst over the retained exp tiles) with a ones
    column appended to v so the softmax denominator falls out of the same
    matmul. Normalization per half: DVE copy of the denominator row to
    partition 0, gpsimd partition_broadcast, reciprocal_approx_fast, one
    multiply per head; the odd head is moved to partitions 64:127 with a
    cross-quadrant stream_shuffle so o_proj runs K=128 per head pair.
  Phase 3: o_proj per token tile (4 accumulating K=128 matmuls, weights
    reused across the two 512-column halves), fp32 partial DMA'd to DRAM.
"""
import sys

for _p in ("/opt/trn_rl_repo", "/root/.axon_site", "/root/.axon_site/_ro/trn_rl_repo",
           "/root/.axon_site/_ro/pypackages"):
    if _p not in sys.path:
        sys.path.append(_p)

import numpy as np

import concourse.bass as bass
import concourse.tile as tile
from concourse import bacc, mybir
from concourse.bass_utils import run_bass_kernel_spmd
from concourse.masks import make_identity

F32 = mybir.dt.float32
F32R = mybir.dt.float32r
BF16 = mybir.dt.bfloat16
P = 128
B, L, C, H, D = 4, 1024, 1024, 16, 64
HC = 8          # heads per core
NPR = HC // 2   # head pairs per core
CG = HC * D     # 512 context channels per core
NT = L // P     # 8 token tiles
NCK = C // P    # 8 contraction tiles
THETA = 50000.0
EPS = 1e-5

_NC_CACHE = {}
# dummy keep-warm matmul counts (fill PE idle so the HAM clock gate stays
# at K=8/8; targets are PSUM slivers cleared by the next start=True group)
WARM1, WARM2, WARM2E, WARM3, WARM3PRE = 0, 0, 0, 0, 0


def _build_nc():
    nc = bacc.Bacc("TRN2", target_bir_lowering=False, debug=False, num_devices=8)

    xT_d = nc.dram_tensor("xT", [P, NT, NCK, P], BF16, kind="ExternalInput")
    wqT_d = nc.dram_tensor("wqT", [C, CG], BF16, kind="ExternalInput")
    wkT_d = nc.dram_tensor("wkT", [C, CG], BF16, kind="ExternalInput")
    wvT_d = nc.dram_tensor("wvT", [C, CG], BF16, kind="ExternalInput")
    woT_d = nc.dram_tensor("woT", [NPR, P, C], BF16, kind="ExternalInput")
    aq_d = nc.dram_tensor("aq", [P, NT, D], BF16, kind="ExternalInput")
    bq_d = nc.dram_tensor("bq", [P, NT, D], BF16, kind="ExternalInput")
    ak_d = nc.dram_tensor("ak", [P, NT, D], BF16, kind="ExternalInput")
    bk_d = nc.dram_tensor("bk", [P, NT, D], BF16, kind="ExternalInput")
    out_d = nc.dram_tensor("out", [L, C], F32, kind="ExternalOutput")

    with tile.TileContext(nc) as tc:
        with (
            tc.tile_pool(name="const", bufs=1) as constp,
            tc.tile_pool(name="w", bufs=1) as wpool,
            tc.tile_pool(name="big", bufs=1) as bigp,
            tc.tile_pool(name="scr", bufs=2) as scrp,
            tc.tile_pool(name="rope", bufs=2) as ropep,
            tc.tile_pool(name="stat", bufs=2) as statp,
            tc.tile_pool(name="exp", bufs=1) as expp,
            tc.tile_pool(name="den", bufs=2) as denp,
            tc.tile_pool(name="fin", bufs=2) as finp,
        ):
            ident = constp.tile([P, P], BF16)
            make_identity(nc, ident)
            eps_t = constp.tile([P, 1], F32)
            nc.vector.memset(eps_t[:], EPS)

            a2_t = constp.tile([P, 2, NT, D], BF16)
            b2_t = constp.tile([P, 2, NT, D], BF16)

            # x resident in SBUF, tile-major. DMA order: x tile 0, all wq,
            # all wk, x tile 1, all wv, x tiles 2..7 — so tile 0's q stats
            # (the head of the DVE pipeline) are ready after ~1.3MB of
            # traffic instead of the full 5MB
            xt_all = bigp.tile([P, NT, NCK, P], BF16)
            wq_t, wk_t, wv_t = [], [], []

            def _w_dmas(lst, nm, d_):
                for ck in range(NCK):
                    t_ = wpool.tile([P, CG], BF16, tag=f"{nm}{ck}", name=f"{nm}{ck}")
                    nc.sync.dma_start(
                        t_[:],
                        d_.ap().rearrange("(k p) o -> p k o", p=P)[:, ck, :])
                    lst.append(t_)

            nc.sync.dma_start(xt_all[:, 0], xT_d.ap()[:, 0])
            _w_dmas(wq_t, "wq", wqT_d)
            _w_dmas(wk_t, "wk", wkT_d)
            nc.sync.dma_start(xt_all[:, 1], xT_d.ap()[:, 1])
            _w_dmas(wv_t, "wv", wvT_d)
            for ti in range(2, NT):
                nc.sync.dma_start(xt_all[:, ti], xT_d.ap()[:, ti])

            nc.sync.dma_start(a2_t[:, 0, :, :], aq_d.ap())
            nc.sync.dma_start(a2_t[:, 1, :, :], ak_d.ap())
            nc.sync.dma_start(b2_t[:, 0, :, :], bq_d.ap())
            nc.sync.dma_start(b2_t[:, 1, :, :], bk_d.ap())

            # v with a ones column appended per head: [s_tile, j, head, 65]
            v_sb = bigp.tile([P, NT, HC, D + 1], BF16)
            nc.vector.memset(
                v_sb[:, :, :, D:D + 1].rearrange("p t h o -> p (t h) o"), 1.0)
            def warm(n, target):
                for _ in range(n):
                    nc.tensor.matmul(target[0:16, 0:16], xt_all[:, 0, 0, 16:32],
                                     xt_all[:, 0, 0, 0:16], start=True, stop=True)


            qT_pack = bigp.tile([P, NPR, L], BF16)
            kT_pack = bigp.tile([P, NPR, L], BF16)
            # ctx packed two heads per 128 partitions: [128, pair, L]
            ctxT2 = bigp.tile([P, NPR, L], BF16)

            # ---------------- Phase 1: QKV + LN + RoPE + transpose ----------
            # processed two token tiles per group: the QKV matmuls and the
            # PSUM-reading ops (reduces, squares, t1) run per tile, the rest
            # of the LN/rope arithmetic runs as merged [P, 2, 2, HC, *] DVE
            # ops to amortize per-op overhead and pipeline drains
            with tc.tile_pool(name="ps1", bufs=2, space="PSUM") as ps1, \
                 tc.tile_pool(name="pst", bufs=2, space="PSUM") as pst:
                for g in range(NT // 2):
                    stats = statp.tile([P, 2, 4, HC], F32, tag="stats")
                    qk_sb = scrp.tile([P, 2, 2, HC, D], BF16, tag="qk_sb")
                    t1 = qk_sb  # LN-apply runs in place on the bf16 copy
                    psqks = []
                    for s in range(2):
                        ti = 2 * g + s
                        psq = ps1.tile([P, CG], F32, tag="psq", name="psq")
                        psk = ps1.tile([P, CG], F32, tag="psk", name="psk")
                        psv = ps1.tile([P, CG], F32, tag="psv", name="psv")
                        psqks.append((psq, psk))
                        for ps_, w_ in ((psq, wq_t), (psk, wk_t), (psv, wv_t)):
                            for ck in range(NCK):
                                nc.tensor.matmul(
                                    ps_[:], xt_all[:, ti, ck, :], w_[ck][:],
                                    start=(ck == 0), stop=(ck == NCK - 1))

                        # v straight to SBUF (bf16); ACT to keep DVE free
                        nc.scalar.copy(
                            v_sb[:, ti, :, 0:D],
                            psv[:].rearrange("p (h d) -> p h d", d=D))

                        # sums / sums of squares per (token, q/k, head);
                        # q/k also copied to bf16 SBUF (ACT) so the PSUM
                        # banks free early and the rope chain runs at the
                        # 2x bf16 DVE rate
                        for i, ps_ in enumerate((psq, psk)):
                            nc.vector.reduce_sum(
                                stats[:, s, 2 * i, :],
                                ps_[:].rearrange("p (h d) -> p h d", d=D),
                                axis=mybir.AxisListType.X)
                            nc.scalar.copy(qk_sb[:, s, i], ps_[:].rearrange(
                                "p (h d) -> p h d", d=D))
                            sq = scrp.tile([P, CG], F32, tag="sq")
                            nc.scalar.square(sq[:], ps_[:])
                            nc.vector.reduce_sum(
                                stats[:, s, 2 * i + 1, :],
                                sq[:].rearrange("p (h d) -> p h d", d=D),
                                axis=mybir.AxisListType.X)
                    mu2 = statp.tile([P, 2, 4, HC], F32, tag="mu2")
                    nc.vector.tensor_scalar_mul(mu2[:], stats[:], 1.0 / D)
                    var = statp.tile([P, 2, 2, HC], F32, tag="var")
                    nc.vector.tensor_mul(var[:], mu2[:, :, 0::2, :], mu2[:, :, 0::2, :])
                    nc.vector.tensor_sub(var[:], mu2[:, :, 1::2, :], var[:])
                    std = statp.tile([P, 2, 2, HC], F32, tag="std")
                    nc.scalar.activation(std[:], var[:],
                                         mybir.ActivationFunctionType.Sqrt,
                                         bias=eps_t[:])
                    inv = statp.tile([P, 2, 2, HC], F32, tag="inv")
                    nc.vector.reciprocal(inv[:], std[:])
                    invh = statp.tile([P, 2, 2, HC], BF16, tag="invh")
                    nc.vector.tensor_copy(invh[:], inv[:])
                    shifth = statp.tile([P, 2, 2, HC], BF16, tag="shifth")
                    nc.vector.tensor_mul(shifth[:], mu2[:, :, 0::2, :], inv[:])

                    h_ = D // 2
                    for s in range(2):
                        ti = 2 * g + s
                        inv_b = invh[:, s].rearrange("p i h -> p i h ()").to_broadcast(
                            (P, 2, HC, D))
                        sh_b = shifth[:, s].rearrange("p i h -> p i h ()").to_broadcast(
                            (P, 2, HC, D))
                        a_b = a2_t[:, :, ti, :].rearrange(
                            "p i d -> p i () d").to_broadcast((P, 2, HC, D))
                        nc.vector.tensor_mul(t1[:, s], t1[:, s], inv_b)
                        nc.vector.tensor_sub(t1[:, s], t1[:, s], sh_b)
                        rope = ropep.tile([P, 2, HC, D], BF16, tag=f"rope{s}")
                        nc.vector.tensor_mul(rope[:], t1[:, s], a_b)
                        r2 = scrp.tile([P, 2, HC, D], BF16, tag=f"r2{s}")
                        nc.vector.tensor_mul(
                            r2[:, :, :, 0:h_], t1[:, s, :, :, h_:D],
                            b2_t[:, :, ti, 0:h_].rearrange(
                                "p i d -> p i () d").to_broadcast((P, 2, HC, h_)))
                        nc.vector.tensor_mul(
                            r2[:, :, :, h_:D], t1[:, s, :, :, 0:h_],
                            b2_t[:, :, ti, h_:D].rearrange(
                                "p i d -> p i () d").to_broadcast((P, 2, HC, h_)))
                        nc.vector.tensor_add(rope[:], rope[:], r2[:])
                        for i, dstpack in ((0, qT_pack), (1, kT_pack)):
                            for pr in range(NPR):
                                ps_t = pst.tile([P, P], BF16)
                                nc.tensor.transpose(
                                    ps_t[:],
                                    rope[:, i, 2 * pr:2 * pr + 2, :].rearrange(
                                        "p h d -> p (h d)"),
                                    ident[:])
                                nc.scalar.copy(dstpack[:, pr, bass.ts(ti, P)], ps_t[:])
                    warm(WARM1, psqks[0][0])

            # o_proj weights early: reuses the per-ck wq slots (dead after
            # phase 1); packed per head pair [128, C] to match ctxT2
            wo_l = []
            for pr in range(NPR):
                wo_p = wpool.tile([P, C], BF16, tag=f"wq{pr}", name=f"wo{pr}")
                nc.sync.dma_start(wo_p[:], woT_d.ap()[pr, :, :])
                wo_l.append(wo_p)

            # ---------------- Phase 2: attention per head pair --------------
            # ctx accumulates per 512-token half into 1-bank PSUM tiles:
            # m=0 interleaved into the scores/exp j-loop, m=1 as a dense
            # matmul burst afterwards (all exp tiles are kept in SBUF).
            # Each half normalizes independently, so no pair-boundary
            # barrier on PSUM and the PE never idles long enough for the
            # HAM clock gate to re-throttle.
            with tc.tile_pool(name="pss", bufs=1, space="PSUM") as pssp, \
                 tc.tile_pool(name="psc", bufs=1, space="PSUM") as pscp:
                shuffle_ident = list(range(32))

                def normalize2(pr, m, pscs):
                    # both heads' chains interleaved so the accumulators
                    # free as early as possible for the next pair
                    dens, rbs = [], []
                    for head in range(2):
                        den = denp.tile([1, 512], F32, tag=f"den{head}{m}",
                                        name=f"den{head}{m}")
                        nc.vector.tensor_copy(den[0:1, :], pscs[head][D:D + 1, :])
                        dens.append(den)
                    for head in range(2):
                        rbr = denp.tile([D, 512], F32, tag=f"rbr{head}{m}",
                                        name=f"rbr{head}{m}")
                        nc.gpsimd.partition_broadcast(rbr[:], dens[head][0:1, :])
                        rb = denp.tile([D, 512], F32, tag=f"rb{head}{m}",
                                       name=f"rb{head}{m}")
                        nc.vector.reciprocal_approx_fast(rb[:], rbr[:])
                        rbs.append(rb)
                    nc.vector.tensor_mul(
                        ctxT2[0:D, pr, bass.ts(m, 512)], pscs[0][0:D, :], rbs[0][:])
                    tmpB = denp.tile([D, 512], BF16, tag=f"tmpB{m}")
                    nc.vector.tensor_mul(tmpB[:], pscs[1][0:D, :], rbs[1][:])
                    nc.vector.stream_shuffle(
                        ctxT2[D:2 * D, pr, bass.ts(m, 512)], tmpB[:],
                        shuffle_ident)

                for pr in range(NPR):
                    hA, hB = 2 * pr, 2 * pr + 1
                    psc0 = [pscp.tile([D + 1, 512], F32, tag=f"pc{h}0",
                                      name=f"pc{h}0") for h in range(2)]
                    psc1 = [pscp.tile([D + 1, 512], F32, tag=f"pc{h}1",
                                      name=f"pc{h}1") for h in range(2)]
                    exps = []
                    for j in range(NT):
                        pss = pssp.tile([P, 2, 2, 512], F32, tag="pss")
                        for m in range(2):
                            for half in range(2):
                                nc.tensor.matmul(
                                    pss[:, half, m, :],
                                    kT_pack[half * D:(half + 1) * D, pr, bass.ts(j, P)],
                                    qT_pack[half * D:(half + 1) * D, pr, bass.ts(m, 512)],
                                    start=True, stop=True)
                        expAB = expp.tile([P, 2, 2, 512], BF16, tag=f"expAB{j}",
                                          name=f"expAB{j}")
                        nc.scalar.activation(expAB[:], pss[:],
                                             mybir.ActivationFunctionType.Exp,
                                             scale=float(D) ** -0.5)
                        exps.append(expAB)
                        # ctx matmuls trail the scores/exp pipeline: m=0 at
                        # lag 2 (so the previous pair's normalize has freed
                        # the m0 accumulator before this hits the PE queue),
                        # m=1 at lag 4 (ditto, and it spreads the exp-tile
                        # WAR reads through the loop instead of a burst at
                        # the pair end that would stall the next pair's exps)
                        def ctx_mm(jc, m, psc_):
                            for head, h in ((0, hA), (1, hB)):
                                nc.tensor.matmul(
                                    psc_[head][:], v_sb[:, jc, h, :],
                                    exps[jc][:, head, m, :],
                                    start=(jc == 0), stop=(jc == NT - 1))
                        if j >= 1:
                            ctx_mm(j - 1, 0, psc0)
                        if j >= 3:
                            ctx_mm(j - 3, 1, psc1)
                    ctx_mm(NT - 1, 0, psc0)
                    for jc in range(NT - 3, NT):
                        ctx_mm(jc, 1, psc1)
                    normalize2(pr, 0, psc0)
                    normalize2(pr, 1, psc1)

            # ---------------- Phase 3: output projection --------------------
            with tc.tile_pool(name="pso", bufs=2, space="PSUM") as psop:
                for ti in range(NT):
                    pso = psop.tile([P, C], F32, name="pso")
                    for pr in range(NPR):
                        for m in range(2):
                            nc.tensor.matmul(
                                pso[:, bass.ts(m, 512)],
                                ctxT2[:, pr, bass.ts(ti, P)],
                                wo_l[pr][:, bass.ts(m, 512)],
                                start=(pr == 0), stop=(pr == NPR - 1))
                    out_sb = finp.tile([P, C], F32, tag="out", name="out_sb")
                    if ti % 2 == 0:
                        nc.scalar.copy(out_sb[:], pso[:])
                    else:
                        nc.vector.tensor_copy(out_sb[:], pso[:])
                    nc.sync.dma_start(out_d.ap()[bass.ts(ti, P), :], out_sb[:])

    nc.compile()
    return nc


def _rope_tables(w, b):
    """A[t,d], B[t,d] with the rotate-half sign folded into B."""
    inv_freq = 1.0 / THETA ** (np.arange(0, D, 2, dtype=np.float64) / D)
    freqs = np.arange(L, dtype=np.float64)[:, None] * inv_freq[None, :]
    freqs = np.concatenate([freqs, freqs], axis=1)           # [L, D]
    cos, sin = np.cos(freqs), np.sin(freqs)
    w = w.astype(np.float64)
    w_rot = np.concatenate([w[D // 2:], w[:D // 2]])
    sgn = np.concatenate([-np.ones(D // 2), np.ones(D // 2)])
    A = (cos * w[None, :]).astype(np.float32)
    Bt = (sin * w_rot[None, :] * sgn[None, :]).astype(np.float32)
    if np.any(b != 0):
        raise NotImplementedError("nonzero qk-norm bias not supported")
    return A, Bt


def kernel(**inputs):
    from ml_dtypes import bfloat16

    x = np.asarray(inputs["q"], dtype=np.float32)
    Wq = np.asarray(inputs["Wq"], dtype=np.float32)
    Wk = np.asarray(inputs["Wk"], dtype=np.float32)
    Wv = np.asarray(inputs["Wv"], dtype=np.float32)
    Wo = np.asarray(inputs["Wo"], dtype=np.float32)
    bo = np.asarray(inputs["bo"], dtype=np.float32)
    assert not np.any(bo != 0), "nonzero output bias not supported"

    Aq, Bq = _rope_tables(np.asarray(inputs["qn_w"], np.float32),
                          np.asarray(inputs["qn_b"], np.float32))
    Ak, Bk = _rope_tables(np.asarray(inputs["kn_w"], np.float32),
                          np.asarray(inputs["kn_b"], np.float32))
    WoT = np.ascontiguousarray(Wo.T)                          # [C(c'), C(o)]

    def _tbl(a):   # [L, D] -> [P, NT, D] (partition-major, contiguous DMA)
        return np.ascontiguousarray(
            a.reshape(NT, P, D).transpose(1, 0, 2)).astype(bfloat16)
    Aqr, Bqr, Akr, Bkr = _tbl(Aq), _tbl(Bq), _tbl(Ak), _tbl(Bk)

    if "nc" not in _NC_CACHE:
        _NC_CACHE["nc"] = _build_nc()
    nc = _NC_CACHE["nc"]

    in_maps = []
    for c in range(8):
        b_, g = c // 2, c % 2
        sl = slice(g * CG, (g + 1) * CG)
        in_maps.append({
            "xT": np.ascontiguousarray(
                x[b_].T.reshape(NCK, P, NT, P).transpose(1, 2, 0, 3)).astype(bfloat16),
            "wqT": np.ascontiguousarray(Wq[sl, :].T).astype(bfloat16),
            "wkT": np.ascontiguousarray(Wk[sl, :].T).astype(bfloat16),
            "wvT": np.ascontiguousarray(Wv[sl, :].T).astype(bfloat16),
            # [pair, 2*D rows (= the pair's context channels), C]
            "woT": np.ascontiguousarray(
                WoT[sl, :].reshape(NPR, P, C)).astype(bfloat16),
            "aq": Aqr, "bq": Bqr, "ak": Akr, "bk": Bkr,
        })

    res = run_bass_kernel_spmd(nc, in_maps, core_ids=list(range(8)))
    # each core wrote its full [L, C] o_proj partial; unshard = sum the two
    # head-group partials per batch
    out = np.empty((B, L, C), dtype=np.float32)
    for b_ in range(B):
        out[b_] = res.results[2 * b_]["out"] + res.results[2 * b_ + 1]["out"]
    return out


# revision 32
# speedup vs baseline: 1.2014x; 1.0159x over previous
"""MultiHeadAttention (qk-LayerNorm + RoPE) Trainium2 kernel, 8 NeuronCores.

Sharding: batch (4) x head-group (2x8 heads). Core c handles batch c//2,
heads 8*(c%2) .. 8*(c%2)+7. Each core computes QKV projections for its
batch restricted to its head group, per-head LayerNorm + rotary embedding,
attention, and a partial output projection over its 512 context channels.
The two partial o_proj results per batch are summed on the host (the
"unshard" step), which keeps the device program collective-free: no NEFF
entry barrier, no ReduceScatter tail.

Dataflow per core (all matmul operands bf16, PSUM accumulation fp32):
  Phase 1: per 128-token tile: QKV projections (bf16, x and per-ck weight
    tiles DMA'd in interleaved order so matmuls start early), LayerNorm
    stats via per-head reductions (merged across two token tiles),
    LN+rope applied in bf16, q/k transposed to [d, t] layout via PE
    transposes (two heads per 128x128 transpose).
  Phase 2: per head pair (row groups 0:64 / 64:128 of the packed q/k
    tiles): scores for both heads concurrently (distinct PE row groups),
    one exp ACT op per j covering both heads [128, 2048], ctx accumulated
    per 512-token half into 1-bank PSUM tiles (m=0 interleaved into the
    j-loop, m=1 as a dense burst over the retained exp tiles) with a ones
    column appended to v so the softmax denominator falls out of the same
    matmul. Normalization per half: DVE copy of the denominator row to
    partition 0, gpsimd partition_broadcast, reciprocal_approx_fast, one
    multiply per head; the odd head is moved to partitions 64:127 with a
    cross-quadrant stream_shuffle so o_proj runs K=128 per head pair.
  Phase 3: o_proj per token tile (4 accumulating K=128 matmuls, weights
    reused across the two 512-column halves), fp32 partial DMA'd to DRAM.
"""
import sys

for _p in ("/opt/trn_rl_repo", "/root/.axon_site", "/root/.axon_site/_ro/trn_rl_repo",
           "/root/.axon_site/_ro/pypackages"):
    if _p not in sys.path:
        sys.path.append(_p)

import numpy as np

import concourse.bass as bass
import concourse.tile as tile
from concourse import bacc, mybir
from concourse.bass_utils import run_bass_kernel_spmd
from concourse.masks import make_identity

F32 = mybir.dt.float32
F32R = mybir.dt.float32r
BF16 = mybir.dt.bfloat16
P = 128
B, L, C, H, D = 4, 1024, 1024, 16, 64
HC = 8          # heads per core
NPR = HC // 2   # head pairs per core
CG = HC * D     # 512 context channels per core
NT = L // P     # 8 token tiles
NCK = C // P    # 8 contraction tiles
THETA = 50000.0
EPS = 1e-5

_NC_CACHE = {}
# dummy keep-warm matmul counts (fill PE idle so the HAM clock gate stays
# at K=8/8; targets are PSUM slivers cleared by the next start=True group)
WARM1, WARM2, WARM2E, WARM3, WARM3PRE = 0, 0, 0, 0, 0


def _build_nc():
    nc = bacc.Bacc("TRN2", target_bir_lowering=False, debug=False, num_devices=8)

    xT_d = nc.dram_tensor("xT", [P, NT, NCK, P], BF16, kind="ExternalInput")
    wqT_d = nc.dram_tensor("wqT", [C, CG], BF16, kind="ExternalInput")
    wkT_d = nc.dram_tensor("wkT", [C, CG], BF16, kind="ExternalInput")
    wvT_d = nc.dram_tensor("wvT", [C, CG], BF16, kind="ExternalInput")
    woT_d = nc.dram_tensor("woT", [NPR, P, C], BF16, kind="ExternalInput")
    aq_d = nc.dram_tensor("aq", [P, NT, D], BF16, kind="ExternalInput")
    bq_d = nc.dram_tensor("bq", [P, NT, D], BF16, kind="ExternalInput")
    ak_d = nc.dram_tensor("ak", [P, NT, D], BF16, kind="ExternalInput")
    bk_d = nc.dram_tensor("bk", [P, NT, D], BF16, kind="ExternalInput")
    out_d = nc.dram_tensor("out", [L, C], BF16, kind="ExternalOutput")

    with tile.TileContext(nc) as tc:
        with (
            tc.tile_pool(name="const", bufs=1) as constp,
            tc.tile_pool(name="w", bufs=1) as wpool,
            tc.tile_pool(name="big", bufs=1) as bigp,
            tc.tile_pool(name="scr", bufs=2) as scrp,
            tc.tile_pool(name="rope", bufs=2) as ropep,
            tc.tile_pool(name="stat", bufs=2) as statp,
            tc.tile_pool(name="exp", bufs=1) as expp,
            tc.tile_pool(name="den", bufs=2) as denp,
            tc.tile_pool(name="fin", bufs=2) as finp,
        ):
            ident = constp.tile([P, P], BF16)
            make_identity(nc, ident)
            eps_t = constp.tile([P, 1], F32)
            nc.vector.memset(eps_t[:], EPS)

            a2_t = constp.tile([P, 2, NT, D], BF16)
            b2_t = constp.tile([P, 2, NT, D], BF16)

            # x resident in SBUF, tile-major. DMA order: x tile 0, all wq,
            # all wk, x tile 1, all wv, x tiles 2..7 — so tile 0's q stats
            # (the head of the DVE pipeline) are ready after ~1.3MB of
            # traffic instead of the full 5MB
            xt_all = bigp.tile([P, NT, NCK, P], BF16)
            wq_t, wk_t, wv_t = [], [], []

            def _w_dmas(lst, nm, d_):
                for ck in range(NCK):
                    t_ = wpool.tile([P, CG], BF16, tag=f"{nm}{ck}", name=f"{nm}{ck}")
                    nc.sync.dma_start(
                        t_[:],
                        d_.ap().rearrange("(k p) o -> p k o", p=P)[:, ck, :])
                    lst.append(t_)

            nc.sync.dma_start(xt_all[:, 0], xT_d.ap()[:, 0])
            _w_dmas(wq_t, "wq", wqT_d)
            _w_dmas(wk_t, "wk", wkT_d)
            nc.sync.dma_start(xt_all[:, 1], xT_d.ap()[:, 1])
            _w_dmas(wv_t, "wv", wvT_d)
            for ti in range(2, NT):
                nc.sync.dma_start(xt_all[:, ti], xT_d.ap()[:, ti])

            nc.sync.dma_start(a2_t[:, 0, :, :], aq_d.ap())
            nc.sync.dma_start(a2_t[:, 1, :, :], ak_d.ap())
            nc.sync.dma_start(b2_t[:, 0, :, :], bq_d.ap())
            nc.sync.dma_start(b2_t[:, 1, :, :], bk_d.ap())

            # v with a ones column appended per head: [s_tile, j, head, 65]
            v_sb = bigp.tile([P, NT, HC, D + 1], BF16)
            nc.vector.memset(
                v_sb[:, :, :, D:D + 1].rearrange("p t h o -> p (t h) o"), 1.0)
            def warm(n, target):
                for _ in range(n):
                    nc.tensor.matmul(target[0:16, 0:16], xt_all[:, 0, 0, 16:32],
                                     xt_all[:, 0, 0, 0:16], start=True, stop=True)


            qT_pack = bigp.tile([P, NPR, L], BF16)
            kT_pack = bigp.tile([P, NPR, L], BF16)
            # ctx packed two heads per 128 partitions: [128, pair, L]
            ctxT2 = bigp.tile([P, NPR, L], BF16)

            # ---------------- Phase 1: QKV + LN + RoPE + transpose ----------
            # processed two token tiles per group: the QKV matmuls and the
            # PSUM-reading ops (reduces, squares, t1) run per tile, the rest
            # of the LN/rope arithmetic runs as merged [P, 2, 2, HC, *] DVE
            # ops to amortize per-op overhead and pipeline drains
            with tc.tile_pool(name="ps1", bufs=2, space="PSUM") as ps1, \
                 tc.tile_pool(name="pst", bufs=2, space="PSUM") as pst:
                for g in range(NT // 2):
                    stats = statp.tile([P, 2, 4, HC], F32, tag="stats")
                    qk_sb = scrp.tile([P, 2, 2, HC, D], BF16, tag="qk_sb")
                    t1 = qk_sb  # LN-apply runs in place on the bf16 copy
                    psqks = []
                    for s in range(2):
                        ti = 2 * g + s
                        psq = ps1.tile([P, CG], F32, tag="psq", name="psq")
                        psk = ps1.tile([P, CG], F32, tag="psk", name="psk")
                        psv = ps1.tile([P, CG], F32, tag="psv", name="psv")
                        psqks.append((psq, psk))
                        for ps_, w_ in ((psq, wq_t), (psk, wk_t), (psv, wv_t)):
                            for ck in range(NCK):
                                nc.tensor.matmul(
                                    ps_[:], xt_all[:, ti, ck, :], w_[ck][:],
                                    start=(ck == 0), stop=(ck == NCK - 1))

                        # v straight to SBUF (bf16); ACT to keep DVE free
                        nc.scalar.copy(
                            v_sb[:, ti, :, 0:D],
                            psv[:].rearrange("p (h d) -> p h d", d=D))

                        # sums / sums of squares per (token, q/k, head);
                        # q/k also copied to bf16 SBUF (ACT) so the PSUM
                        # banks free early and the rope chain runs at the
                        # 2x bf16 DVE rate
                        for i, ps_ in enumerate((psq, psk)):
                            nc.vector.reduce_sum(
                                stats[:, s, 2 * i, :],
                                ps_[:].rearrange("p (h d) -> p h d", d=D),
                                axis=mybir.AxisListType.X)
                            nc.scalar.copy(qk_sb[:, s, i], ps_[:].rearrange(
                                "p (h d) -> p h d", d=D))
                            sq = scrp.tile([P, CG], F32, tag="sq")
                            nc.scalar.square(sq[:], ps_[:])
                            nc.vector.reduce_sum(
                                stats[:, s, 2 * i + 1, :],
                                sq[:].rearrange("p (h d) -> p h d", d=D),
                                axis=mybir.AxisListType.X)
                    mu2 = statp.tile([P, 2, 4, HC], F32, tag="mu2")
                    nc.vector.tensor_scalar_mul(mu2[:], stats[:], 1.0 / D)
                    var = statp.tile([P, 2, 2, HC], F32, tag="var")
                    nc.vector.tensor_mul(var[:], mu2[:, :, 0::2, :], mu2[:, :, 0::2, :])
                    nc.vector.tensor_sub(var[:], mu2[:, :, 1::2, :], var[:])
                    std = statp.tile([P, 2, 2, HC], F32, tag="std")
                    nc.scalar.activation(std[:], var[:],
                                         mybir.ActivationFunctionType.Sqrt,
                                         bias=eps_t[:])
                    inv = statp.tile([P, 2, 2, HC], F32, tag="inv")
                    nc.vector.reciprocal(inv[:], std[:])
                    invh = statp.tile([P, 2, 2, HC], BF16, tag="invh")
                    nc.vector.tensor_copy(invh[:], inv[:])
                    shifth = statp.tile([P, 2, 2, HC], BF16, tag="shifth")
                    nc.vector.tensor_mul(shifth[:], mu2[:, :, 0::2, :], inv[:])

                    h_ = D // 2
                    for s in range(2):
                        ti = 2 * g + s
                        inv_b = invh[:, s].rearrange("p i h -> p i h ()").to_broadcast(
                            (P, 2, HC, D))
                        sh_b = shifth[:, s].rearrange("p i h -> p i h ()").to_broadcast(
                            (P, 2, HC, D))
                        a_b = a2_t[:, :, ti, :].rearrange(
                            "p i d -> p i () d").to_broadcast((P, 2, HC, D))
                        nc.vector.tensor_mul(t1[:, s], t1[:, s], inv_b)
                        nc.vector.tensor_sub(t1[:, s], t1[:, s], sh_b)
                        rope = ropep.tile([P, 2, HC, D], BF16, tag=f"rope{s}")
                        nc.vector.tensor_mul(rope[:], t1[:, s], a_b)
                        r2 = scrp.tile([P, 2, HC, D], BF16, tag=f"r2{s}")
                        nc.vector.tensor_mul(
                            r2[:, :, :, 0:h_], t1[:, s, :, :, h_:D],
                            b2_t[:, :, ti, 0:h_].rearrange(
                                "p i d -> p i () d").to_broadcast((P, 2, HC, h_)))
                        nc.vector.tensor_mul(
                            r2[:, :, :, h_:D], t1[:, s, :, :, 0:h_],
                            b2_t[:, :, ti, h_:D].rearrange(
                                "p i d -> p i () d").to_broadcast((P, 2, HC, h_)))
                        nc.vector.tensor_add(rope[:], rope[:], r2[:])
                        for i, dstpack in ((0, qT_pack), (1, kT_pack)):
                            for pr in range(NPR):
                                ps_t = pst.tile([P, P], BF16)
                                nc.tensor.transpose(
                                    ps_t[:],
                                    rope[:, i, 2 * pr:2 * pr + 2, :].rearrange(
                                        "p h d -> p (h d)"),
                                    ident[:])
                                nc.scalar.copy(dstpack[:, pr, bass.ts(ti, P)], ps_t[:])
                    warm(WARM1, psqks[0][0])

            # o_proj weights early: reuses the per-ck wq slots (dead after
            # phase 1); packed per head pair [128, C] to match ctxT2
            wo_l = []
            for pr in range(NPR):
                wo_p = wpool.tile([P, C], BF16, tag=f"wq{pr}", name=f"wo{pr}")
                nc.sync.dma_start(wo_p[:], woT_d.ap()[pr, :, :])
                wo_l.append(wo_p)

            # ---------------- Phase 2: attention per head pair --------------
            # ctx accumulates per 512-token half into 1-bank PSUM tiles:
            # m=0 interleaved into the scores/exp j-loop, m=1 as a dense
            # matmul burst afterwards (all exp tiles are kept in SBUF).
            # Each half normalizes independently, so no pair-boundary
            # barrier on PSUM and the PE never idles long enough for the
            # HAM clock gate to re-throttle.
            with tc.tile_pool(name="pss", bufs=1, space="PSUM") as pssp, \
                 tc.tile_pool(name="psc", bufs=1, space="PSUM") as pscp:
                shuffle_ident = list(range(32))

                def normalize2(pr, m, pscs):
                    # both heads' chains interleaved so the accumulators
                    # free as early as possible for the next pair
                    dens, rbs = [], []
                    for head in range(2):
                        den = denp.tile([1, 512], F32, tag=f"den{head}{m}",
                                        name=f"den{head}{m}")
                        nc.vector.tensor_copy(den[0:1, :], pscs[head][D:D + 1, :])
                        dens.append(den)
                    for head in range(2):
                        rbr = denp.tile([D, 512], F32, tag=f"rbr{head}{m}",
                                        name=f"rbr{head}{m}")
                        nc.gpsimd.partition_broadcast(rbr[:], dens[head][0:1, :])
                        rb = denp.tile([D, 512], F32, tag=f"rb{head}{m}",
                                       name=f"rb{head}{m}")
                        nc.vector.reciprocal_approx_fast(rb[:], rbr[:])
                        rbs.append(rb)
                    nc.vector.tensor_mul(
                        ctxT2[0:D, pr, bass.ts(m, 512)], pscs[0][0:D, :], rbs[0][:])
                    tmpB = denp.tile([D, 512], BF16, tag=f"tmpB{m}")
                    nc.vector.tensor_mul(tmpB[:], pscs[1][0:D, :], rbs[1][:])
                    nc.vector.stream_shuffle(
                        ctxT2[D:2 * D, pr, bass.ts(m, 512)], tmpB[:],
                        shuffle_ident)

                for pr in range(NPR):
                    hA, hB = 2 * pr, 2 * pr + 1
                    psc0 = [pscp.tile([D + 1, 512], F32, tag=f"pc{h}0",
                                      name=f"pc{h}0") for h in range(2)]
                    psc1 = [pscp.tile([D + 1, 512], F32, tag=f"pc{h}1",
                                      name=f"pc{h}1") for h in range(2)]
                    exps = []
                    for j in range(NT):
                        pss = pssp.tile([P, 2, 2, 512], F32, tag="pss")
                        for m in range(2):
                            for half in range(2):
                                nc.tensor.matmul(
                                    pss[:, half, m, :],
                                    kT_pack[half * D:(half + 1) * D, pr, bass.ts(j, P)],
                                    qT_pack[half * D:(half + 1) * D, pr, bass.ts(m, 512)],
                                    start=True, stop=True)
                        expAB = expp.tile([P, 2, 2, 512], BF16, tag=f"expAB{j}",
                                          name=f"expAB{j}")
                        nc.scalar.activation(expAB[:], pss[:],
                                             mybir.ActivationFunctionType.Exp,
                                             scale=float(D) ** -0.5)
                        exps.append(expAB)
                        # ctx matmuls trail the scores/exp pipeline: m=0 at
                        # lag 2 (so the previous pair's normalize has freed
                        # the m0 accumulator before this hits the PE queue),
                        # m=1 at lag 4 (ditto, and it spreads the exp-tile
                        # WAR reads through the loop instead of a burst at
                        # the pair end that would stall the next pair's exps)
                        def ctx_mm(jc, m, psc_):
                            for head, h in ((0, hA), (1, hB)):
                                nc.tensor.matmul(
                                    psc_[head][:], v_sb[:, jc, h, :],
                                    exps[jc][:, head, m, :],
                                    start=(jc == 0), stop=(jc == NT - 1))
                        if j >= 1:
                            ctx_mm(j - 1, 0, psc0)
                        if j >= 3:
                            ctx_mm(j - 3, 1, psc1)
                    ctx_mm(NT - 1, 0, psc0)
                    for jc in range(NT - 3, NT):
                        ctx_mm(jc, 1, psc1)
                    normalize2(pr, 0, psc0)
                    normalize2(pr, 1, psc1)

            # ---------------- Phase 3: output projection --------------------
            with tc.tile_pool(name="pso", bufs=2, space="PSUM") as psop:
                for ti in range(NT):
                    pso = psop.tile([P, C], F32, name="pso")
                    for pr in range(NPR):
                        for m in range(2):
                            nc.tensor.matmul(
                                pso[:, bass.ts(m, 512)],
                                ctxT2[:, pr, bass.ts(ti, P)],
                                wo_l[pr][:, bass.ts(m, 512)],
                                start=(pr == 0), stop=(pr == NPR - 1))
                    out_sb = finp.tile([P, C], BF16, tag="out", name="out_sb")
                    if ti % 2 == 0:
                        nc.scalar.copy(out_sb[:], pso[:])
                    else:
                        nc.vector.tensor_copy(out_sb[:], pso[:])
                    nc.sync.dma_start(out_d.ap()[bass.ts(ti, P), :], out_sb[:])

    nc.compile()
    return nc


def _rope_tables(w, b):
    """A[t,d], B[t,d] with the rotate-half sign folded into B."""
    inv_freq = 1.0 / THETA ** (np.arange(0, D, 2, dtype=np.float64) / D)
    freqs = np.arange(L, dtype=np.float64)[:, None] * inv_freq[None, :]
    freqs = np.concatenate([freqs, freqs], axis=1)           # [L, D]
    cos, sin = np.cos(freqs), np.sin(freqs)
    w = w.astype(np.float64)
    w_rot = np.concatenate([w[D // 2:], w[:D // 2]])
    sgn = np.concatenate([-np.ones(D // 2), np.ones(D // 2)])
    A = (cos * w[None, :]).astype(np.float32)
    Bt = (sin * w_rot[None, :] * sgn[None, :]).astype(np.float32)
    if np.any(b != 0):
        raise NotImplementedError("nonzero qk-norm bias not supported")
    return A, Bt


def kernel(**inputs):
    from ml_dtypes import bfloat16

    x = np.asarray(inputs["q"], dtype=np.float32)
    Wq = np.asarray(inputs["Wq"], dtype=np.float32)
    Wk = np.asarray(inputs["Wk"], dtype=np.float32)
    Wv = np.asarray(inputs["Wv"], dtype=np.float32)
    Wo = np.asarray(inputs["Wo"], dtype=np.float32)
    bo = np.asarray(inputs["bo"], dtype=np.float32)
    assert not np.any(bo != 0), "nonzero output bias not supported"

    Aq, Bq = _rope_tables(np.asarray(inputs["qn_w"], np.float32),
                          np.asarray(inputs["qn_b"], np.float32))
    Ak, Bk = _rope_tables(np.asarray(inputs["kn_w"], np.float32),
                          np.asarray(inputs["kn_b"], np.float32))
    WoT = np.ascontiguousarray(Wo.T)                          # [C(c'), C(o)]

    def _tbl(a):   # [L, D] -> [P, NT, D] (partition-major, contiguous DMA)
        return np.ascontiguousarray(
            a.reshape(NT, P, D).transpose(1, 0, 2)).astype(bfloat16)
    Aqr, Bqr, Akr, Bkr = _tbl(Aq), _tbl(Bq), _tbl(Ak), _tbl(Bk)

    if "nc" not in _NC_CACHE:
        _NC_CACHE["nc"] = _build_nc()
    nc = _NC_CACHE["nc"]

    in_maps = []
    for c in range(8):
        b_, g = c // 2, c % 2
        sl = slice(g * CG, (g + 1) * CG)
        in_maps.append({
            "xT": np.ascontiguousarray(
                x[b_].T.reshape(NCK, P, NT, P).transpose(1, 2, 0, 3)).astype(bfloat16),
            "wqT": np.ascontiguousarray(Wq[sl, :].T).astype(bfloat16),
            "wkT": np.ascontiguousarray(Wk[sl, :].T).astype(bfloat16),
            "wvT": np.ascontiguousarray(Wv[sl, :].T).astype(bfloat16),
            # [pair, 2*D rows (= the pair's context channels), C]
            "woT": np.ascontiguousarray(
                WoT[sl, :].reshape(NPR, P, C)).astype(bfloat16),
            "aq": Aqr, "bq": Bqr, "ak": Akr, "bk": Bkr,
        })

    res = run_bass_kernel_spmd(nc, in_maps, core_ids=list(range(8)))
    # each core wrote its full [L, C] o_proj partial; unshard = sum the two
    # head-group partials per batch
    out = np.empty((B, L, C), dtype=np.float32)
    for b_ in range(B):
        out[b_] = (res.results[2 * b_]["out"].astype(np.float32)
                   + res.results[2 * b_ + 1]["out"].astype(np.float32))
    return out


# revision 33
# speedup vs baseline: 1.2455x; 1.0367x over previous
"""MultiHeadAttention (qk-LayerNorm + RoPE) Trainium2 kernel, 8 NeuronCores.

Sharding: batch (4) x head-group (2x8 heads). Core c handles batch c//2,
heads 8*(c%2) .. 8*(c%2)+7. Each core computes QKV projections for its
batch restricted to its head group, per-head LayerNorm + rotary embedding,
attention, and a partial output projection over its 512 context channels.
The two partial o_proj results per batch are summed on the host (the
"unshard" step), which keeps the device program collective-free: no NEFF
entry barrier, no ReduceScatter tail.

Dataflow per core (all matmul operands bf16, PSUM accumulation fp32):
  Phase 1: per 128-token tile: QKV projections (bf16, x and per-ck weight
    tiles DMA'd in interleaved order so matmuls start early), LayerNorm
    stats via per-head reductions (merged across two token tiles),
    LN+rope applied in bf16, q/k transposed to [d, t] layout via PE
    transposes (two heads per 128x128 transpose).
  Phase 2: per head pair (row groups 0:64 / 64:128 of the packed q/k
    tiles): scores for both heads concurrently (distinct PE row groups),
    one exp ACT op per j covering both heads [128, 2048], ctx accumulated
    per 512-token half into 1-bank PSUM tiles (m=0 interleaved into the
    j-loop, m=1 as a dense burst over the retained exp tiles) with a ones
    column appended to v so the softmax denominator falls out of the same
    matmul. Normalization per half: DVE copy of the denominator row to
    partition 0, gpsimd partition_broadcast, reciprocal_approx_fast, one
    multiply per head; the odd head is moved to partitions 64:127 with a
    cross-quadrant stream_shuffle so o_proj runs K=128 per head pair.
  Phase 3: o_proj per token tile (4 accumulating K=128 matmuls, weights
    reused across the two 512-column halves), fp32 partial DMA'd to DRAM.
"""
import sys

for _p in ("/opt/trn_rl_repo", "/root/.axon_site", "/root/.axon_site/_ro/trn_rl_repo",
           "/root/.axon_site/_ro/pypackages"):
    if _p not in sys.path:
        sys.path.append(_p)

import numpy as np

import concourse.bass as bass
import concourse.tile as tile
from concourse import bacc, mybir
from concourse.bass_utils import run_bass_kernel_spmd
from concourse.masks import make_identity

F32 = mybir.dt.float32
F32R = mybir.dt.float32r
BF16 = mybir.dt.bfloat16
P = 128
B, L, C, H, D = 4, 1024, 1024, 16, 64
HC = 8          # heads per core
NPR = HC // 2   # head pairs per core
CG = HC * D     # 512 context channels per core
NT = L // P     # 8 token tiles
NCK = C // P    # 8 contraction tiles
THETA = 50000.0
EPS = 1e-5

_NC_CACHE = {}
# dummy keep-warm matmul counts (fill PE idle so the HAM clock gate stays
# at K=8/8; targets are PSUM slivers cleared by the next start=True group)
WARM1, WARM2, WARM2E, WARM3, WARM3PRE = 0, 0, 0, 0, 0


def _build_nc():
    nc = bacc.Bacc("TRN2", target_bir_lowering=False, debug=False, num_devices=8)

    xT_d = nc.dram_tensor("xT", [P, NT, NCK, P], BF16, kind="ExternalInput")
    wqT_d = nc.dram_tensor("wqT", [C, CG], BF16, kind="ExternalInput")
    wkT_d = nc.dram_tensor("wkT", [C, CG], BF16, kind="ExternalInput")
    wvT_d = nc.dram_tensor("wvT", [C, CG], BF16, kind="ExternalInput")
    woT_d = nc.dram_tensor("woT", [NPR, P, C], BF16, kind="ExternalInput")
    aq_d = nc.dram_tensor("aq", [P, NT, D], BF16, kind="ExternalInput")
    bq_d = nc.dram_tensor("bq", [P, NT, D], BF16, kind="ExternalInput")
    ak_d = nc.dram_tensor("ak", [P, NT, D], BF16, kind="ExternalInput")
    bk_d = nc.dram_tensor("bk", [P, NT, D], BF16, kind="ExternalInput")
    out_d = nc.dram_tensor("out", [L, C], BF16, kind="ExternalOutput")

    with tile.TileContext(nc) as tc:
        with (
            tc.tile_pool(name="const", bufs=1) as constp,
            tc.tile_pool(name="w", bufs=1) as wpool,
            tc.tile_pool(name="big", bufs=1) as bigp,
            tc.tile_pool(name="scr", bufs=2) as scrp,
            tc.tile_pool(name="rope", bufs=2) as ropep,
            tc.tile_pool(name="stat", bufs=2) as statp,
            tc.tile_pool(name="exp", bufs=1) as expp,
            tc.tile_pool(name="den", bufs=2) as denp,
            tc.tile_pool(name="fin", bufs=2) as finp,
        ):
            ident = constp.tile([P, P], BF16)
            make_identity(nc, ident)
            eps_t = constp.tile([P, 1], F32)
            nc.vector.memset(eps_t[:], EPS)

            a2_t = constp.tile([P, 2, NT, D], BF16)
            b2_t = constp.tile([P, 2, NT, D], BF16)

            # x resident in SBUF, tile-major. DMA order: x tile 0, all wq,
            # all wk, x tile 1, all wv, x tiles 2..7 — so tile 0's q stats
            # (the head of the DVE pipeline) are ready after ~1.3MB of
            # traffic instead of the full 5MB
            xt_all = bigp.tile([P, NT, NCK, P], BF16)
            wq_t, wk_t, wv_t = [], [], []

            def _w_dmas(lst, nm, d_):
                for ck in range(NCK):
                    t_ = wpool.tile([P, CG], BF16, tag=f"{nm}{ck}", name=f"{nm}{ck}")
                    nc.sync.dma_start(
                        t_[:],
                        d_.ap().rearrange("(k p) o -> p k o", p=P)[:, ck, :])
                    lst.append(t_)

            nc.sync.dma_start(xt_all[:, 0], xT_d.ap()[:, 0])
            _w_dmas(wq_t, "wq", wqT_d)
            _w_dmas(wk_t, "wk", wkT_d)
            nc.sync.dma_start(xt_all[:, 1], xT_d.ap()[:, 1])
            _w_dmas(wv_t, "wv", wvT_d)
            for ti in range(2, NT):
                nc.sync.dma_start(xt_all[:, ti], xT_d.ap()[:, ti])

            nc.sync.dma_start(a2_t[:, 0, :, :], aq_d.ap())
            nc.sync.dma_start(a2_t[:, 1, :, :], ak_d.ap())
            nc.sync.dma_start(b2_t[:, 0, :, :], bq_d.ap())
            nc.sync.dma_start(b2_t[:, 1, :, :], bk_d.ap())

            # v with a ones column appended per head: [s_tile, j, head, 65]
            v_sb = bigp.tile([P, NT, HC, D + 1], BF16)
            nc.vector.memset(
                v_sb[:, :, :, D:D + 1].rearrange("p t h o -> p (t h) o"), 1.0)
            def warm(n, target):
                for _ in range(n):
                    nc.tensor.matmul(target[0:16, 0:16], xt_all[:, 0, 0, 16:32],
                                     xt_all[:, 0, 0, 0:16], start=True, stop=True)


            qT_pack = bigp.tile([P, NPR, L], BF16)
            kT_pack = bigp.tile([P, NPR, L], BF16)
            # ctx packed two heads per 128 partitions: [128, pair, L]
            ctxT2 = bigp.tile([P, NPR, L], BF16)

            # ---------------- Phase 1: QKV + LN + RoPE + transpose ----------
            # processed two token tiles per group: the QKV matmuls and the
            # PSUM-reading ops (reduces, squares, t1) run per tile, the rest
            # of the LN/rope arithmetic runs as merged [P, 2, 2, HC, *] DVE
            # ops to amortize per-op overhead and pipeline drains
            with tc.tile_pool(name="ps1", bufs=2, space="PSUM") as ps1, \
                 tc.tile_pool(name="pst", bufs=2, space="PSUM") as pst:
                for g in range(NT // 2):
                    stats = statp.tile([P, 2, 4, HC], F32, tag="stats")
                    qk_sb = scrp.tile([P, 2, 2, HC, D], BF16, tag="qk_sb")
                    t1 = qk_sb  # LN-apply runs in place on the bf16 copy
                    psqks = []
                    for s in range(2):
                        ti = 2 * g + s
                        psq = ps1.tile([P, CG], F32, tag="psq", name="psq")
                        psk = ps1.tile([P, CG], F32, tag="psk", name="psk")
                        psv = ps1.tile([P, CG], F32, tag="psv", name="psv")
                        psqks.append((psq, psk))
                        for ps_, w_ in ((psq, wq_t), (psk, wk_t), (psv, wv_t)):
                            for ck in range(NCK):
                                nc.tensor.matmul(
                                    ps_[:], xt_all[:, ti, ck, :], w_[ck][:],
                                    start=(ck == 0), stop=(ck == NCK - 1))

                        # v straight to SBUF (bf16); ACT to keep DVE free
                        nc.scalar.copy(
                            v_sb[:, ti, :, 0:D],
                            psv[:].rearrange("p (h d) -> p h d", d=D))

                        # q/k copied to bf16 SBUF (ACT); PSUM frees right
                        # after the copy, and all stats (sum / sum of
                        # squares) run as merged q+k ops on the copy
                        for i, ps_ in enumerate((psq, psk)):
                            nc.scalar.copy(qk_sb[:, s, i], ps_[:].rearrange(
                                "p (h d) -> p h d", d=D))
                        nc.vector.reduce_sum(
                            stats[:, s, 0:2, :].rearrange("p a h -> p (a h)"),
                            qk_sb[:, s].rearrange("p i h d -> p (i h) d"),
                            axis=mybir.AxisListType.X)
                        sq = scrp.tile([P, 2, HC, D], BF16, tag="sq")
                        nc.scalar.square(sq[:], qk_sb[:, s])
                        nc.vector.reduce_sum(
                            stats[:, s, 2:4, :].rearrange("p a h -> p (a h)"),
                            sq[:].rearrange("p i h d -> p (i h) d"),
                            axis=mybir.AxisListType.X)
                    mu2 = statp.tile([P, 2, 4, HC], F32, tag="mu2")
                    nc.vector.tensor_scalar_mul(mu2[:], stats[:], 1.0 / D)
                    var = statp.tile([P, 2, 2, HC], F32, tag="var")
                    nc.vector.tensor_mul(var[:], mu2[:, :, 0:2, :], mu2[:, :, 0:2, :])
                    nc.vector.tensor_sub(var[:], mu2[:, :, 2:4, :], var[:])
                    std = statp.tile([P, 2, 2, HC], F32, tag="std")
                    nc.scalar.activation(std[:], var[:],
                                         mybir.ActivationFunctionType.Sqrt,
                                         bias=eps_t[:])
                    inv = statp.tile([P, 2, 2, HC], F32, tag="inv")
                    nc.vector.reciprocal(inv[:], std[:])
                    invh = statp.tile([P, 2, 2, HC], BF16, tag="invh")
                    nc.vector.tensor_copy(invh[:], inv[:])
                    shifth = statp.tile([P, 2, 2, HC], BF16, tag="shifth")
                    nc.vector.tensor_mul(shifth[:], mu2[:, :, 0:2, :], inv[:])

                    h_ = D // 2
                    for s in range(2):
                        ti = 2 * g + s
                        inv_b = invh[:, s].rearrange("p i h -> p i h ()").to_broadcast(
                            (P, 2, HC, D))
                        sh_b = shifth[:, s].rearrange("p i h -> p i h ()").to_broadcast(
                            (P, 2, HC, D))
                        a_b = a2_t[:, :, ti, :].rearrange(
                            "p i d -> p i () d").to_broadcast((P, 2, HC, D))
                        nc.vector.tensor_mul(t1[:, s], t1[:, s], inv_b)
                        nc.vector.tensor_sub(t1[:, s], t1[:, s], sh_b)
                        rope = ropep.tile([P, 2, HC, D], BF16, tag=f"rope{s}")
                        nc.vector.tensor_mul(rope[:], t1[:, s], a_b)
                        r2 = scrp.tile([P, 2, HC, D], BF16, tag=f"r2{s}")
                        nc.vector.tensor_mul(
                            r2[:, :, :, 0:h_], t1[:, s, :, :, h_:D],
                            b2_t[:, :, ti, 0:h_].rearrange(
                                "p i d -> p i () d").to_broadcast((P, 2, HC, h_)))
                        nc.vector.tensor_mul(
                            r2[:, :, :, h_:D], t1[:, s, :, :, 0:h_],
                            b2_t[:, :, ti, h_:D].rearrange(
                                "p i d -> p i () d").to_broadcast((P, 2, HC, h_)))
                        nc.vector.tensor_add(rope[:], rope[:], r2[:])
                        for i, dstpack in ((0, qT_pack), (1, kT_pack)):
                            for pr in range(NPR):
                                ps_t = pst.tile([P, P], BF16)
                                nc.tensor.transpose(
                                    ps_t[:],
                                    rope[:, i, 2 * pr:2 * pr + 2, :].rearrange(
                                        "p h d -> p (h d)"),
                                    ident[:])
                                nc.scalar.copy(dstpack[:, pr, bass.ts(ti, P)], ps_t[:])
                    warm(WARM1, psqks[0][0])

            # o_proj weights early: reuses the per-ck wq slots (dead after
            # phase 1); packed per head pair [128, C] to match ctxT2
            wo_l = []
            for pr in range(NPR):
                wo_p = wpool.tile([P, C], BF16, tag=f"wq{pr}", name=f"wo{pr}")
                nc.sync.dma_start(wo_p[:], woT_d.ap()[pr, :, :])
                wo_l.append(wo_p)

            # ---------------- Phase 2: attention per head pair --------------
            # ctx accumulates per 512-token half into 1-bank PSUM tiles:
            # m=0 interleaved into the scores/exp j-loop, m=1 as a dense
            # matmul burst afterwards (all exp tiles are kept in SBUF).
            # Each half normalizes independently, so no pair-boundary
            # barrier on PSUM and the PE never idles long enough for the
            # HAM clock gate to re-throttle.
            with tc.tile_pool(name="pss", bufs=1, space="PSUM") as pssp, \
                 tc.tile_pool(name="psc", bufs=1, space="PSUM") as pscp:
                shuffle_ident = list(range(32))

                def normalize2(pr, m, pscs):
                    # both heads' chains interleaved so the accumulators
                    # free as early as possible for the next pair
                    dens, rbs = [], []
                    for head in range(2):
                        den = denp.tile([1, 512], F32, tag=f"den{head}{m}",
                                        name=f"den{head}{m}")
                        nc.vector.tensor_copy(den[0:1, :], pscs[head][D:D + 1, :])
                        dens.append(den)
                    for head in range(2):
                        rbr = denp.tile([D, 512], F32, tag=f"rbr{head}{m}",
                                        name=f"rbr{head}{m}")
                        nc.gpsimd.partition_broadcast(rbr[:], dens[head][0:1, :])
                        rb = denp.tile([D, 512], F32, tag=f"rb{head}{m}",
                                       name=f"rb{head}{m}")
                        nc.vector.reciprocal_approx_fast(rb[:], rbr[:])
                        rbs.append(rb)
                    nc.vector.tensor_mul(
                        ctxT2[0:D, pr, bass.ts(m, 512)], pscs[0][0:D, :], rbs[0][:])
                    tmpB = denp.tile([D, 512], BF16, tag=f"tmpB{m}")
                    nc.vector.tensor_mul(tmpB[:], pscs[1][0:D, :], rbs[1][:])
                    nc.vector.stream_shuffle(
                        ctxT2[D:2 * D, pr, bass.ts(m, 512)], tmpB[:],
                        shuffle_ident)

                for pr in range(NPR):
                    hA, hB = 2 * pr, 2 * pr + 1
                    psc0 = [pscp.tile([D + 1, 512], F32, tag=f"pc{h}0",
                                      name=f"pc{h}0") for h in range(2)]
                    psc1 = [pscp.tile([D + 1, 512], F32, tag=f"pc{h}1",
                                      name=f"pc{h}1") for h in range(2)]
                    exps = []
                    for j in range(NT):
                        pss = pssp.tile([P, 2, 2, 512], F32, tag="pss")
                        for m in range(2):
                            for half in range(2):
                                nc.tensor.matmul(
                                    pss[:, half, m, :],
                                    kT_pack[half * D:(half + 1) * D, pr, bass.ts(j, P)],
                                    qT_pack[half * D:(half + 1) * D, pr, bass.ts(m, 512)],
                                    start=True, stop=True)
                        expAB = expp.tile([P, 2, 2, 512], BF16, tag=f"expAB{j}",
                                          name=f"expAB{j}")
                        nc.scalar.activation(expAB[:], pss[:],
                                             mybir.ActivationFunctionType.Exp,
                                             scale=float(D) ** -0.5)
                        exps.append(expAB)
                        # ctx matmuls trail the scores/exp pipeline: m=0 at
                        # lag 2 (so the previous pair's normalize has freed
                        # the m0 accumulator before this hits the PE queue),
                        # m=1 at lag 4 (ditto, and it spreads the exp-tile
                        # WAR reads through the loop instead of a burst at
                        # the pair end that would stall the next pair's exps)
                        def ctx_mm(jc, m, psc_):
                            for head, h in ((0, hA), (1, hB)):
                                nc.tensor.matmul(
                                    psc_[head][:], v_sb[:, jc, h, :],
                                    exps[jc][:, head, m, :],
                                    start=(jc == 0), stop=(jc == NT - 1))
                        if j >= 1:
                            ctx_mm(j - 1, 0, psc0)
                        if j >= 3:
                            ctx_mm(j - 3, 1, psc1)
                    ctx_mm(NT - 1, 0, psc0)
                    for jc in range(NT - 3, NT):
                        ctx_mm(jc, 1, psc1)
                    normalize2(pr, 0, psc0)
                    normalize2(pr, 1, psc1)

            # ---------------- Phase 3: output projection --------------------
            with tc.tile_pool(name="pso", bufs=2, space="PSUM") as psop:
                for ti in range(NT):
                    pso = psop.tile([P, C], F32, name="pso")
                    for pr in range(NPR):
                        for m in range(2):
                            nc.tensor.matmul(
                                pso[:, bass.ts(m, 512)],
                                ctxT2[:, pr, bass.ts(ti, P)],
                                wo_l[pr][:, bass.ts(m, 512)],
                                start=(pr == 0), stop=(pr == NPR - 1))
                    out_sb = finp.tile([P, C], BF16, tag="out", name="out_sb")
                    nc.vector.tensor_copy(out_sb[:], pso[:])
                    nc.sync.dma_start(out_d.ap()[bass.ts(ti, P), :], out_sb[:])

    nc.compile()
    return nc


def _rope_tables(w, b):
    """A[t,d], B[t,d] with the rotate-half sign folded into B."""
    inv_freq = 1.0 / THETA ** (np.arange(0, D, 2, dtype=np.float64) / D)
    freqs = np.arange(L, dtype=np.float64)[:, None] * inv_freq[None, :]
    freqs = np.concatenate([freqs, freqs], axis=1)           # [L, D]
    cos, sin = np.cos(freqs), np.sin(freqs)
    w = w.astype(np.float64)
    w_rot = np.concatenate([w[D // 2:], w[:D // 2]])
    sgn = np.concatenate([-np.ones(D // 2), np.ones(D // 2)])
    A = (cos * w[None, :]).astype(np.float32)
    Bt = (sin * w_rot[None, :] * sgn[None, :]).astype(np.float32)
    if np.any(b != 0):
        raise NotImplementedError("nonzero qk-norm bias not supported")
    return A, Bt


def kernel(**inputs):
    from ml_dtypes import bfloat16

    x = np.asarray(inputs["q"], dtype=np.float32)
    Wq = np.asarray(inputs["Wq"], dtype=np.float32)
    Wk = np.asarray(inputs["Wk"], dtype=np.float32)
    Wv = np.asarray(inputs["Wv"], dtype=np.float32)
    Wo = np.asarray(inputs["Wo"], dtype=np.float32)
    bo = np.asarray(inputs["bo"], dtype=np.float32)
    assert not np.any(bo != 0), "nonzero output bias not supported"

    Aq, Bq = _rope_tables(np.asarray(inputs["qn_w"], np.float32),
                          np.asarray(inputs["qn_b"], np.float32))
    Ak, Bk = _rope_tables(np.asarray(inputs["kn_w"], np.float32),
                          np.asarray(inputs["kn_b"], np.float32))
    WoT = np.ascontiguousarray(Wo.T)                          # [C(c'), C(o)]

    def _tbl(a):   # [L, D] -> [P, NT, D] (partition-major, contiguous DMA)
        return np.ascontiguousarray(
            a.reshape(NT, P, D).transpose(1, 0, 2)).astype(bfloat16)
    Aqr, Bqr, Akr, Bkr = _tbl(Aq), _tbl(Bq), _tbl(Ak), _tbl(Bk)

    if "nc" not in _NC_CACHE:
        _NC_CACHE["nc"] = _build_nc()
    nc = _NC_CACHE["nc"]

    in_maps = []
    for c in range(8):
        b_, g = c // 2, c % 2
        sl = slice(g * CG, (g + 1) * CG)
        in_maps.append({
            "xT": np.ascontiguousarray(
                x[b_].T.reshape(NCK, P, NT, P).transpose(1, 2, 0, 3)).astype(bfloat16),
            "wqT": np.ascontiguousarray(Wq[sl, :].T).astype(bfloat16),
            "wkT": np.ascontiguousarray(Wk[sl, :].T).astype(bfloat16),
            "wvT": np.ascontiguousarray(Wv[sl, :].T).astype(bfloat16),
            # [pair, 2*D rows (= the pair's context channels), C]
            "woT": np.ascontiguousarray(
                WoT[sl, :].reshape(NPR, P, C)).astype(bfloat16),
            "aq": Aqr, "bq": Bqr, "ak": Akr, "bk": Bkr,
        })

    res = run_bass_kernel_spmd(nc, in_maps, core_ids=list(range(8)))
    # each core wrote its full [L, C] o_proj partial; unshard = sum the two
    # head-group partials per batch
    out = np.empty((B, L, C), dtype=np.float32)
    for b_ in range(B):
        out[b_] = (res.results[2 * b_]["out"].astype(np.float32)
                   + res.results[2 * b_ + 1]["out"].astype(np.float32))
    return out


# revision 34
# speedup vs baseline: 1.2544x; 1.0072x over previous
"""MultiHeadAttention (qk-LayerNorm + RoPE) Trainium2 kernel, 8 NeuronCores.

Sharding: batch (4) x head-group (2x8 heads). Core c handles batch c//2,
heads 8*(c%2) .. 8*(c%2)+7. Each core computes QKV projections for its
batch restricted to its head group, per-head LayerNorm + rotary embedding,
attention, and a partial output projection over its 512 context channels.
The two partial o_proj results per batch are summed on the host (the
"unshard" step), which keeps the device program collective-free: no NEFF
entry barrier, no ReduceScatter tail.

Dataflow per core (all matmul operands bf16, PSUM accumulation fp32):
  Phase 1: per 128-token tile: QKV projections (bf16, x and per-ck weight
    tiles DMA'd in interleaved order so matmuls start early), LayerNorm
    stats via per-head reductions (merged across two token tiles),
    LN+rope applied in bf16, q/k transposed to [d, t] layout via PE
    transposes (two heads per 128x128 transpose).
  Phase 2: per head pair (row groups 0:64 / 64:128 of the packed q/k
    tiles): scores for both heads concurrently (distinct PE row groups),
    one exp ACT op per j covering both heads [128, 2048], ctx accumulated
    per 512-token half into 1-bank PSUM tiles (m=0 interleaved into the
    j-loop, m=1 as a dense burst over the retained exp tiles) with a ones
    column appended to v so the softmax denominator falls out of the same
    matmul. Normalization per half: DVE copy of the denominator row to
    partition 0, gpsimd partition_broadcast, reciprocal_approx_fast, one
    multiply per head; the odd head is moved to partitions 64:127 with a
    cross-quadrant stream_shuffle so o_proj runs K=128 per head pair.
  Phase 3: o_proj per token tile (4 accumulating K=128 matmuls, weights
    reused across the two 512-column halves), fp32 partial DMA'd to DRAM.
"""
import sys

for _p in ("/opt/trn_rl_repo", "/root/.axon_site", "/root/.axon_site/_ro/trn_rl_repo",
           "/root/.axon_site/_ro/pypackages"):
    if _p not in sys.path:
        sys.path.append(_p)

import numpy as np

import concourse.bass as bass
import concourse.tile as tile
from concourse import bacc, mybir
from concourse.bass_utils import run_bass_kernel_spmd
from concourse.masks import make_identity

F32 = mybir.dt.float32
F32R = mybir.dt.float32r
BF16 = mybir.dt.bfloat16
P = 128
B, L, C, H, D = 4, 1024, 1024, 16, 64
HC = 8          # heads per core
NPR = HC // 2   # head pairs per core
CG = HC * D     # 512 context channels per core
NT = L // P     # 8 token tiles
NCK = C // P    # 8 contraction tiles
THETA = 50000.0
EPS = 1e-5

_NC_CACHE = {}
# dummy keep-warm matmul counts (fill PE idle so the HAM clock gate stays
# at K=8/8; targets are PSUM slivers cleared by the next start=True group)
WARM1, WARM2, WARM2E, WARM3, WARM3PRE = 0, 0, 0, 0, 0


def _build_nc():
    nc = bacc.Bacc("TRN2", target_bir_lowering=False, debug=False, num_devices=8)

    xT_d = nc.dram_tensor("xT", [P, NT, NCK, P], BF16, kind="ExternalInput")
    wqT_d = nc.dram_tensor("wqT", [C, CG], BF16, kind="ExternalInput")
    wkT_d = nc.dram_tensor("wkT", [C, CG], BF16, kind="ExternalInput")
    wvT_d = nc.dram_tensor("wvT", [C, CG], BF16, kind="ExternalInput")
    woT_d = nc.dram_tensor("woT", [NPR, P, C], BF16, kind="ExternalInput")
    aq_d = nc.dram_tensor("aq", [P, NT, D], BF16, kind="ExternalInput")
    bq_d = nc.dram_tensor("bq", [P, NT, D], BF16, kind="ExternalInput")
    ak_d = nc.dram_tensor("ak", [P, NT, D], BF16, kind="ExternalInput")
    bk_d = nc.dram_tensor("bk", [P, NT, D], BF16, kind="ExternalInput")
    out_d = nc.dram_tensor("out", [L, C], BF16, kind="ExternalOutput")

    with tile.TileContext(nc) as tc:
        with (
            tc.tile_pool(name="const", bufs=1) as constp,
            tc.tile_pool(name="w", bufs=1) as wpool,
            tc.tile_pool(name="big", bufs=1) as bigp,
            tc.tile_pool(name="scr", bufs=2) as scrp,
            tc.tile_pool(name="rope", bufs=2) as ropep,
            tc.tile_pool(name="stat", bufs=2) as statp,
            tc.tile_pool(name="exp", bufs=1) as expp,
            tc.tile_pool(name="den", bufs=2) as denp,
            tc.tile_pool(name="fin", bufs=2) as finp,
        ):
            ident = constp.tile([P, P], BF16)
            make_identity(nc, ident)
            eps_t = constp.tile([P, 1], F32)
            nc.vector.memset(eps_t[:], EPS)

            a2_t = constp.tile([P, 2, NT, D], BF16)
            b2_t = constp.tile([P, 2, NT, D], BF16)

            # x resident in SBUF, tile-major. DMA order: x tile 0, all wq,
            # all wk, x tile 1, all wv, x tiles 2..7 — so tile 0's q stats
            # (the head of the DVE pipeline) are ready after ~1.3MB of
            # traffic instead of the full 5MB
            xt_all = bigp.tile([P, NT, NCK, P], BF16)
            wq_t, wk_t, wv_t = [], [], []

            def _w_dmas(lst, nm, d_):
                for ck in range(NCK):
                    t_ = wpool.tile([P, CG], BF16, tag=f"{nm}{ck}", name=f"{nm}{ck}")
                    nc.sync.dma_start(
                        t_[:],
                        d_.ap().rearrange("(k p) o -> p k o", p=P)[:, ck, :])
                    lst.append(t_)

            nc.sync.dma_start(xt_all[:, 0], xT_d.ap()[:, 0])
            _w_dmas(wq_t, "wq", wqT_d)
            _w_dmas(wk_t, "wk", wkT_d)
            nc.sync.dma_start(xt_all[:, 1], xT_d.ap()[:, 1])
            _w_dmas(wv_t, "wv", wvT_d)
            for ti in range(2, NT):
                nc.sync.dma_start(xt_all[:, ti], xT_d.ap()[:, ti])

            nc.sync.dma_start(a2_t[:, 0, :, :], aq_d.ap())
            nc.sync.dma_start(a2_t[:, 1, :, :], ak_d.ap())
            nc.sync.dma_start(b2_t[:, 0, :, :], bq_d.ap())
            nc.sync.dma_start(b2_t[:, 1, :, :], bk_d.ap())

            # v with a ones column appended per head: [s_tile, j, head, 65]
            v_sb = bigp.tile([P, NT, HC, D + 1], BF16)
            nc.vector.memset(
                v_sb[:, :, :, D:D + 1].rearrange("p t h o -> p (t h) o"), 1.0)
            def warm(n, target):
                for _ in range(n):
                    nc.tensor.matmul(target[0:16, 0:16], xt_all[:, 0, 0, 16:32],
                                     xt_all[:, 0, 0, 0:16], start=True, stop=True)


            qT_pack = bigp.tile([P, NPR, L], BF16)
            kT_pack = bigp.tile([P, NPR, L], BF16)
            # ctx packed two heads per 128 partitions: [128, pair, L]
            ctxT2 = bigp.tile([P, NPR, L], BF16)

            # ---------------- Phase 1: QKV + LN + RoPE + transpose ----------
            # processed two token tiles per group: the QKV matmuls and the
            # PSUM-reading ops (reduces, squares, t1) run per tile, the rest
            # of the LN/rope arithmetic runs as merged [P, 2, 2, HC, *] DVE
            # ops to amortize per-op overhead and pipeline drains
            with tc.tile_pool(name="ps1", bufs=2, space="PSUM") as ps1, \
                 tc.tile_pool(name="pst", bufs=2, space="PSUM") as pst:
                for g in range(NT // 2):
                    stats = statp.tile([P, 2, 4, HC], F32, tag="stats")
                    qk_sb = scrp.tile([P, 2, 2, HC, D], BF16, tag="qk_sb")
                    t1 = qk_sb  # LN-apply runs in place on the bf16 copy
                    psqks = []
                    for s in range(2):
                        ti = 2 * g + s
                        psq = ps1.tile([P, CG], F32, tag="psq", name="psq")
                        psk = ps1.tile([P, CG], F32, tag="psk", name="psk")
                        psv = ps1.tile([P, CG], F32, tag="psv", name="psv")
                        psqks.append((psq, psk))
                        for ps_, w_ in ((psq, wq_t), (psk, wk_t), (psv, wv_t)):
                            for ck in range(NCK):
                                nc.tensor.matmul(
                                    ps_[:], xt_all[:, ti, ck, :], w_[ck][:],
                                    start=(ck == 0), stop=(ck == NCK - 1))

                        # v straight to SBUF (bf16); ACT to keep DVE free
                        nc.scalar.copy(
                            v_sb[:, ti, :, 0:D],
                            psv[:].rearrange("p (h d) -> p h d", d=D))

                        # q/k copied to bf16 SBUF (ACT); PSUM frees right
                        # after the copy and the stats reductions run on the
                        # copy, per tensor so the q-side chain starts as soon
                        # as the q projection lands (k weights arrive later)
                        sq = scrp.tile([P, 2, HC, D], BF16, tag="sq")
                        for i, ps_ in enumerate((psq, psk)):
                            nc.scalar.copy(qk_sb[:, s, i], ps_[:].rearrange(
                                "p (h d) -> p h d", d=D))
                            nc.vector.reduce_sum(
                                stats[:, s, i, :], qk_sb[:, s, i],
                                axis=mybir.AxisListType.X)
                            nc.scalar.square(sq[:, i], qk_sb[:, s, i])
                            nc.vector.reduce_sum(
                                stats[:, s, 2 + i, :], sq[:, i],
                                axis=mybir.AxisListType.X)
                    mu2 = statp.tile([P, 2, 4, HC], F32, tag="mu2")
                    nc.vector.tensor_scalar_mul(mu2[:], stats[:], 1.0 / D)
                    var = statp.tile([P, 2, 2, HC], F32, tag="var")
                    nc.vector.tensor_mul(var[:], mu2[:, :, 0:2, :], mu2[:, :, 0:2, :])
                    nc.vector.tensor_sub(var[:], mu2[:, :, 2:4, :], var[:])
                    std = statp.tile([P, 2, 2, HC], F32, tag="std")
                    nc.scalar.activation(std[:], var[:],
                                         mybir.ActivationFunctionType.Sqrt,
                                         bias=eps_t[:])
                    inv = statp.tile([P, 2, 2, HC], F32, tag="inv")
                    nc.vector.reciprocal(inv[:], std[:])
                    invh = statp.tile([P, 2, 2, HC], BF16, tag="invh")
                    nc.vector.tensor_copy(invh[:], inv[:])
                    shifth = statp.tile([P, 2, 2, HC], BF16, tag="shifth")
                    nc.vector.tensor_mul(shifth[:], mu2[:, :, 0:2, :], inv[:])

                    h_ = D // 2
                    for s in range(2):
                        ti = 2 * g + s
                        inv_b = invh[:, s].rearrange("p i h -> p i h ()").to_broadcast(
                            (P, 2, HC, D))
                        sh_b = shifth[:, s].rearrange("p i h -> p i h ()").to_broadcast(
                            (P, 2, HC, D))
                        a_b = a2_t[:, :, ti, :].rearrange(
                            "p i d -> p i () d").to_broadcast((P, 2, HC, D))
                        nc.vector.tensor_mul(t1[:, s], t1[:, s], inv_b)
                        nc.vector.tensor_sub(t1[:, s], t1[:, s], sh_b)
                        rope = ropep.tile([P, 2, HC, D], BF16, tag=f"rope{s}")
                        nc.vector.tensor_mul(rope[:], t1[:, s], a_b)
                        r2 = scrp.tile([P, 2, HC, D], BF16, tag=f"r2{s}")
                        nc.vector.tensor_mul(
                            r2[:, :, :, 0:h_], t1[:, s, :, :, h_:D],
                            b2_t[:, :, ti, 0:h_].rearrange(
                                "p i d -> p i () d").to_broadcast((P, 2, HC, h_)))
                        nc.vector.tensor_mul(
                            r2[:, :, :, h_:D], t1[:, s, :, :, 0:h_],
                            b2_t[:, :, ti, h_:D].rearrange(
                                "p i d -> p i () d").to_broadcast((P, 2, HC, h_)))
                        nc.vector.tensor_add(rope[:], rope[:], r2[:])
                        for i, dstpack in ((0, qT_pack), (1, kT_pack)):
                            for pr in range(NPR):
                                ps_t = pst.tile([P, P], BF16)
                                nc.tensor.transpose(
                                    ps_t[:],
                                    rope[:, i, 2 * pr:2 * pr + 2, :].rearrange(
                                        "p h d -> p (h d)"),
                                    ident[:])
                                nc.scalar.copy(dstpack[:, pr, bass.ts(ti, P)], ps_t[:])
                    warm(WARM1, psqks[0][0])

            # o_proj weights early: reuses the per-ck wq slots (dead after
            # phase 1); packed per head pair [128, C] to match ctxT2
            wo_l = []
            for pr in range(NPR):
                wo_p = wpool.tile([P, C], BF16, tag=f"wq{pr}", name=f"wo{pr}")
                nc.sync.dma_start(wo_p[:], woT_d.ap()[pr, :, :])
                wo_l.append(wo_p)

            # ---------------- Phase 2: attention per head pair --------------
            # ctx accumulates per 512-token half into 1-bank PSUM tiles:
            # m=0 interleaved into the scores/exp j-loop, m=1 as a dense
            # matmul burst afterwards (all exp tiles are kept in SBUF).
            # Each half normalizes independently, so no pair-boundary
            # barrier on PSUM and the PE never idles long enough for the
            # HAM clock gate to re-throttle.
            with tc.tile_pool(name="pss", bufs=1, space="PSUM") as pssp, \
                 tc.tile_pool(name="psc", bufs=1, space="PSUM") as pscp:
                shuffle_ident = list(range(32))

                def normalize2(pr, m, pscs):
                    # both heads' chains interleaved so the accumulators
                    # free as early as possible for the next pair
                    dens, rbs = [], []
                    for head in range(2):
                        den = denp.tile([1, 512], F32, tag=f"den{head}{m}",
                                        name=f"den{head}{m}")
                        nc.vector.tensor_copy(den[0:1, :], pscs[head][D:D + 1, :])
                        dens.append(den)
                    for head in range(2):
                        rbr = denp.tile([D, 512], F32, tag=f"rbr{head}{m}",
                                        name=f"rbr{head}{m}")
                        nc.gpsimd.partition_broadcast(rbr[:], dens[head][0:1, :])
                        rb = denp.tile([D, 512], F32, tag=f"rb{head}{m}",
                                       name=f"rb{head}{m}")
                        nc.vector.reciprocal_approx_fast(rb[:], rbr[:])
                        rbs.append(rb)
                    nc.vector.tensor_mul(
                        ctxT2[0:D, pr, bass.ts(m, 512)], pscs[0][0:D, :], rbs[0][:])
                    tmpB = denp.tile([D, 512], BF16, tag=f"tmpB{m}")
                    nc.vector.tensor_mul(tmpB[:], pscs[1][0:D, :], rbs[1][:])
                    nc.vector.stream_shuffle(
                        ctxT2[D:2 * D, pr, bass.ts(m, 512)], tmpB[:],
                        shuffle_ident)

                for pr in range(NPR):
                    hA, hB = 2 * pr, 2 * pr + 1
                    psc0 = [pscp.tile([D + 1, 512], F32, tag=f"pc{h}0",
                                      name=f"pc{h}0") for h in range(2)]
                    psc1 = [pscp.tile([D + 1, 512], F32, tag=f"pc{h}1",
                                      name=f"pc{h}1") for h in range(2)]
                    exps = []
                    for j in range(NT):
                        pss = pssp.tile([P, 2, 2, 512], F32, tag="pss")
                        for m in range(2):
                            for half in range(2):
                                nc.tensor.matmul(
                                    pss[:, half, m, :],
                                    kT_pack[half * D:(half + 1) * D, pr, bass.ts(j, P)],
                                    qT_pack[half * D:(half + 1) * D, pr, bass.ts(m, 512)],
                                    start=True, stop=True)
                        expAB = expp.tile([P, 2, 2, 512], BF16, tag=f"expAB{j}",
                                          name=f"expAB{j}")
                        nc.scalar.activation(expAB[:], pss[:],
                                             mybir.ActivationFunctionType.Exp,
                                             scale=float(D) ** -0.5)
                        exps.append(expAB)
                        # ctx matmuls trail the scores/exp pipeline: m=0 at
                        # lag 2 (so the previous pair's normalize has freed
                        # the m0 accumulator before this hits the PE queue),
                        # m=1 at lag 4 (ditto, and it spreads the exp-tile
                        # WAR reads through the loop instead of a burst at
                        # the pair end that would stall the next pair's exps)
                        def ctx_mm(jc, m, psc_):
                            for head, h in ((0, hA), (1, hB)):
                                nc.tensor.matmul(
                                    psc_[head][:], v_sb[:, jc, h, :],
                                    exps[jc][:, head, m, :],
                                    start=(jc == 0), stop=(jc == NT - 1))
                        if j >= 1:
                            ctx_mm(j - 1, 0, psc0)
                        if j >= 3:
                            ctx_mm(j - 3, 1, psc1)
                    ctx_mm(NT - 1, 0, psc0)
                    for jc in range(NT - 3, NT):
                        ctx_mm(jc, 1, psc1)
                    normalize2(pr, 0, psc0)
                    normalize2(pr, 1, psc1)

            # ---------------- Phase 3: output projection --------------------
            with tc.tile_pool(name="pso", bufs=2, space="PSUM") as psop:
                for ti in range(NT):
                    pso = psop.tile([P, C], F32, name="pso")
                    for pr in range(NPR):
                        for m in range(2):
                            nc.tensor.matmul(
                                pso[:, bass.ts(m, 512)],
                                ctxT2[:, pr, bass.ts(ti, P)],
                                wo_l[pr][:, bass.ts(m, 512)],
                                start=(pr == 0), stop=(pr == NPR - 1))
                    out_sb = finp.tile([P, C], BF16, tag="out", name="out_sb")
                    nc.vector.tensor_copy(out_sb[:], pso[:])
                    nc.sync.dma_start(out_d.ap()[bass.ts(ti, P), :], out_sb[:])

    nc.compile()
    return nc


def _rope_tables(w, b):
    """A[t,d], B[t,d] with the rotate-half sign folded into B."""
    inv_freq = 1.0 / THETA ** (np.arange(0, D, 2, dtype=np.float64) / D)
    freqs = np.arange(L, dtype=np.float64)[:, None] * inv_freq[None, :]
    freqs = np.concatenate([freqs, freqs], axis=1)           # [L, D]
    cos, sin = np.cos(freqs), np.sin(freqs)
    w = w.astype(np.float64)
    w_rot = np.concatenate([w[D // 2:], w[:D // 2]])
    sgn = np.concatenate([-np.ones(D // 2), np.ones(D // 2)])
    A = (cos * w[None, :]).astype(np.float32)
    Bt = (sin * w_rot[None, :] * sgn[None, :]).astype(np.float32)
    if np.any(b != 0):
        raise NotImplementedError("nonzero qk-norm bias not supported")
    return A, Bt


def kernel(**inputs):
    from ml_dtypes import bfloat16

    x = np.asarray(inputs["q"], dtype=np.float32)
    Wq = np.asarray(inputs["Wq"], dtype=np.float32)
    Wk = np.asarray(inputs["Wk"], dtype=np.float32)
    Wv = np.asarray(inputs["Wv"], dtype=np.float32)
    Wo = np.asarray(inputs["Wo"], dtype=np.float32)
    bo = np.asarray(inputs["bo"], dtype=np.float32)
    assert not np.any(bo != 0), "nonzero output bias not supported"

    Aq, Bq = _rope_tables(np.asarray(inputs["qn_w"], np.float32),
                          np.asarray(inputs["qn_b"], np.float32))
    Ak, Bk = _rope_tables(np.asarray(inputs["kn_w"], np.float32),
                          np.asarray(inputs["kn_b"], np.float32))
    WoT = np.ascontiguousarray(Wo.T)                          # [C(c'), C(o)]

    def _tbl(a):   # [L, D] -> [P, NT, D] (partition-major, contiguous DMA)
        return np.ascontiguousarray(
            a.reshape(NT, P, D).transpose(1, 0, 2)).astype(bfloat16)
    Aqr, Bqr, Akr, Bkr = _tbl(Aq), _tbl(Bq), _tbl(Ak), _tbl(Bk)

    if "nc" not in _NC_CACHE:
        _NC_CACHE["nc"] = _build_nc()
    nc = _NC_CACHE["nc"]

    in_maps = []
    for c in range(8):
        b_, g = c // 2, c % 2
        sl = slice(g * CG, (g + 1) * CG)
        in_maps.append({
            "xT": np.ascontiguousarray(
                x[b_].T.reshape(NCK, P, NT, P).transpose(1, 2, 0, 3)).astype(bfloat16),
            "wqT": np.ascontiguousarray(Wq[sl, :].T).astype(bfloat16),
            "wkT": np.ascontiguousarray(Wk[sl, :].T).astype(bfloat16),
            "wvT": np.ascontiguousarray(Wv[sl, :].T).astype(bfloat16),
            # [pair, 2*D rows (= the pair's context channels), C]
            "woT": np.ascontiguousarray(
                WoT[sl, :].reshape(NPR, P, C)).astype(bfloat16),
            "aq": Aqr, "bq": Bqr, "ak": Akr, "bk": Bkr,
        })

    res = run_bass_kernel_spmd(nc, in_maps, core_ids=list(range(8)))
    # each core wrote its full [L, C] o_proj partial; unshard = sum the two
    # head-group partials per batch
    out = np.empty((B, L, C), dtype=np.float32)
    for b_ in range(B):
        out[b_] = (res.results[2 * b_]["out"].astype(np.float32)
                   + res.results[2 * b_ + 1]["out"].astype(np.float32))
    return out
